# revision 9
# baseline (speedup 1.0000x reference)
"""Trainium2 Bass kernel for nn_LlamaAttention_17085379903943.

LlamaAttention with LoRA on q/v projections + RoPE, B=2, S=2048, H=2048,
nh=16, hd=128, LoRA rank 16.

Sharding: tensor-parallel over heads across 8 NeuronCores. Each core owns 2
heads (a 256-wide slice of the qkv projection output space) and computes
q/k/v projections (+LoRA), RoPE, and full-sequence softmax attention for its
heads over both batch elements. The per-core context output ctxT
[256, 4096] (features x tokens) is exchanged with a jax-level all_to_all so
that each core ends up with all 2048 context features for a 512-token slice,
then a second bass kernel applies the output projection.  Host code only
slices/transposes/casts inputs and concatenates the 8 output slices.

Compute layout notes (PE matmul computes out = lhsT.T @ rhs, contraction on
the partition dim):
 - hs is fed transposed (hsT [H, T]) so h sits on partitions for the
   projections.  q/k are produced transposed per head (qT/kT [hd, t]) which
   is exactly the layout attention needs; v is produced natural [t, d].
 - scoresT [k, q] = kT_chunk.T @ qT, exp via ScalarE (scale=1/sqrt(hd)
   folded in) with bf16 probsT output.
 - ctx [q, d] accumulates probsT_chunk.T @ [v | ones]: the appended ones
   column makes column 128 the softmax denominator for free; eviction
   multiplies by its reciprocal (per-partition scalar broadcast).
 - RoPE is applied on the [d, t] layout: the rotate-half partition shift is
   done with two small PSUM->SBUF DMAs, sign folded into the sin table.
"""

import math
from contextlib import ExitStack

import numpy as np
import ml_dtypes

import concourse.bass as bass
import concourse.mybir as mybir
import concourse.tile as tile
from concourse import bacc
from concourse import bass2jax
from concourse.masks import make_identity

N_CORES = 8
B = 2
S = 2048
H = 2048
NH = 16
HD = 128
R = 16
LORA_SCALING = 2.0
ROPE_BASE = 10000.0

H_LOC = NH // N_CORES  # heads per core = 2
F_LOC = H_LOC * HD  # feature slice per core = 256
HC = H // 128  # h chunks = 16
T = B * S  # tokens = 4096
T_SLC = T // N_CORES  # output token slice per core = 512

BF16 = mybir.dt.bfloat16
F32 = mybir.dt.float32
np_bf16 = ml_dtypes.bfloat16

AF = mybir.ActivationFunctionType


def build_attn(s=S, n_cores=N_CORES, lowering=True):
    """Kernel 1: qkv projections + LoRA + RoPE + attention -> ctxT."""
    t_all = B * s
    TB = s // 512  # 512-wide t blocks per batch (proj moving blocks)
    TT = s // 128  # 128-wide t tiles per batch
    QG = s // 512  # 512-wide q groups
    KC = s // 128  # k chunks

    nc = bacc.Bacc(
        None,
        num_devices=n_cores,
        target_bir_lowering=lowering,
        enable_partition_id=False,
    )

    hsT = nc.dram_tensor("hsT", [H, t_all], BF16, kind="ExternalInput")
    wqT = nc.dram_tensor("wqT", [H, F_LOC], BF16, kind="ExternalInput")
    wkT = nc.dram_tensor("wkT", [H, F_LOC], BF16, kind="ExternalInput")
    wvT = nc.dram_tensor("wvT", [H, F_LOC], BF16, kind="ExternalInput")
    abT = nc.dram_tensor("abT", [H, 2 * R], BF16, kind="ExternalInput")
    bqT = nc.dram_tensor("bqT", [R, F_LOC], BF16, kind="ExternalInput")
    bvT = nc.dram_tensor("bvT", [R, F_LOC], BF16, kind="ExternalInput")
    cosT = nc.dram_tensor("cosT", [HD, s], F32, kind="ExternalInput")
    sinT = nc.dram_tensor("sinT", [HD, s], F32, kind="ExternalInput")
    ctxT = nc.dram_tensor("ctxT", [F_LOC, t_all], BF16, kind="ExternalOutput")

    inv_sqrt_hd = 1.0 / math.sqrt(HD)

    with tile.TileContext(nc) as tc, ExitStack() as ctx:
        const = ctx.enter_context(tc.tile_pool(name="const", bufs=1))
        wpool = ctx.enter_context(tc.tile_pool(name="wpool", bufs=1))
        hs_pool = ctx.enter_context(tc.tile_pool(name="hs_pool", bufs=HC))
        qk_pool = ctx.enter_context(tc.tile_pool(name="qk_pool", bufs=1))
        v_pool = ctx.enter_context(tc.tile_pool(name="v_pool", bufs=1))
        lora_pool = ctx.enter_context(tc.tile_pool(name="lora_pool", bufs=1))
        probs_pool = ctx.enter_context(tc.tile_pool(name="probs_pool", bufs=2))
        tmp_pool = ctx.enter_context(tc.tile_pool(name="tmp_pool", bufs=3))
        out_pool = ctx.enter_context(tc.tile_pool(name="out_pool", bufs=4))
        psum = ctx.enter_context(tc.tile_pool(name="psum", bufs=1, space="PSUM"))

        # --- constants ---
        ident = const.tile([128, 128], BF16)
        make_identity(nc, ident)
        cos_sb = const.tile([HD, s], F32)
        nc.sync.dma_start(out=cos_sb, in_=cosT[:, :])
        sin_sb = const.tile([HD, s], F32)
        nc.sync.dma_start(out=sin_sb, in_=sinT[:, :])

        # --- weights (resident) ---
        # [h, o] views with h split into 16 chunks of 128 partitions
        wq_sb = wpool.tile([128, HC, F_LOC], BF16)
        nc.sync.dma_start(out=wq_sb, in_=wqT.rearrange("(c p) o -> p c o", p=128))
        wk_sb = wpool.tile([128, HC, F_LOC], BF16)
        nc.sync.dma_start(out=wk_sb, in_=wkT.rearrange("(c p) o -> p c o", p=128))
        wv_sb = wpool.tile([128, HC, F_LOC], BF16)
        nc.sync.dma_start(out=wv_sb, in_=wvT.rearrange("(c p) o -> p c o", p=128))
        ab_sb = wpool.tile([128, HC, 2 * R], BF16)
        nc.sync.dma_start(out=ab_sb, in_=abT.rearrange("(c p) o -> p c o", p=128))
        bq_sb = wpool.tile([R, F_LOC], BF16)
        nc.sync.dma_start(out=bq_sb, in_=bqT[:, :])
        bv_sb = wpool.tile([R, F_LOC], BF16)
        nc.sync.dma_start(out=bv_sb, in_=bvT[:, :])

        hsT_v = hsT.rearrange("(c p) t -> c p t", p=128)

        for b in range(B):
            tok0 = b * s

            # --- load hs chunks for this batch ---
            hs_tiles = []
            for c in range(HC):
                hst = hs_pool.tile([128, s], BF16, name=f"hst{c}", tag="hst")
                nc.sync.dma_start(out=hst, in_=hsT_v[c, :, tok0 : tok0 + s])
                hs_tiles.append(hst)

            # --- LoRA down-projection: [2R, s] = abT.T @ hsT ---
            lora_sb = lora_pool.tile([2 * R, s], BF16, tag="lora")
            lorav_sb = lora_pool.tile([R, s], BF16, tag="lorav")
            for tb in range(TB):
                ps_l = psum.tile([2 * R, 512], F32, tag="proj", bufs=2)
                for c in range(HC):
                    nc.tensor.matmul(
                        ps_l,
                        lhsT=ab_sb[:, c, :],
                        rhs=hs_tiles[c][:, tb * 512 : (tb + 1) * 512],
                        start=(c == 0),
                        stop=(c == HC - 1),
                    )
                nc.vector.tensor_copy(lora_sb[:, tb * 512 : (tb + 1) * 512], ps_l)
            nc.sync.dma_start(out=lorav_sb[:, :], in_=lora_sb[R : 2 * R, :])

            # --- q/k projections (+ q LoRA) with fused RoPE eviction ---
            qT_sb = qk_pool.tile([128, H_LOC, s], BF16, name="qT_sb", tag="qT")
            kT_sb = qk_pool.tile([128, H_LOC, s], BF16, name="kT_sb", tag="kT")
            for w_sb, b_lora_sb, dest in (
                (wq_sb, bq_sb, qT_sb),
                (wk_sb, None, kT_sb),
            ):
                for ot in range(H_LOC):
                    for tb in range(TB):
                        tsl = slice(tb * 512, (tb + 1) * 512)
                        ps = psum.tile([128, 512], F32, tag="proj", bufs=2)
                        for c in range(HC):
                            nc.tensor.matmul(
                                ps,
                                lhsT=w_sb[:, c, ot * 128 : (ot + 1) * 128],
                                rhs=hs_tiles[c][:, tsl],
                                start=(c == 0),
                                stop=(c == HC - 1 and b_lora_sb is None),
                            )
                        if b_lora_sb is not None:
                            nc.tensor.matmul(
                                ps,
                                lhsT=b_lora_sb[:, ot * 128 : (ot + 1) * 128],
                                rhs=lora_sb[0:R, tsl],
                                start=False,
                                stop=True,
                            )
                        # RoPE: dest = qf*cos + shift(qf)*sin'  (sign in table)
                        qf = tmp_pool.tile([128, 512], F32, tag="qf")
                        nc.scalar.copy(qf, ps)
                        shift = tmp_pool.tile([128, 512], F32, tag="shift")
                        nc.sync.dma_start(out=shift[0:64, :], in_=qf[64:128, :])
                        nc.sync.dma_start(out=shift[64:128, :], in_=qf[0:64, :])
                        t1 = tmp_pool.tile([128, 512], F32, tag="t1")
                        nc.vector.tensor_mul(t1, shift, sin_sb[:, tsl])
                        t2 = tmp_pool.tile([128, 512], F32, tag="t2")
                        nc.vector.tensor_mul(t2, qf, cos_sb[:, tsl])
                        nc.vector.tensor_add(dest[:, ot, tsl], t1, t2)

            # --- v projection (natural [t, o] layout) + ones column ---
            v_sb = v_pool.tile([128, TT, H_LOC, HD + 1], BF16, name="v_sb", tag="v")
            nc.vector.memset(v_sb[:, :, :, HD : HD + 1], 1.0)
            for tt in range(TT):
                ps_v = psum.tile([128, F_LOC], F32, tag="proj", bufs=2)
                for c in range(HC):
                    nc.tensor.matmul(
                        ps_v,
                        lhsT=hs_tiles[c][:, tt * 128 : (tt + 1) * 128],
                        rhs=wv_sb[:, c, :],
                        start=(c == 0),
                        stop=False,
                    )
                nc.tensor.matmul(
                    ps_v,
                    lhsT=lorav_sb[:, tt * 128 : (tt + 1) * 128],
                    rhs=bv_sb,
                    start=False,
                    stop=True,
                )
                for h in range(H_LOC):
                    nc.vector.tensor_copy(
                        v_sb[:, tt, h, 0:HD], ps_v[:, h * 128 : (h + 1) * 128]
                    )

            # --- attention per head ---
            for h in range(H_LOC):
                for qg in range(QG):
                    qsl = slice(qg * 512, (qg + 1) * 512)
                    pt = probs_pool.tile([128, KC, 512], BF16, name="pt", tag="probs")
                    for kc in range(KC):
                        ps_s = psum.tile([128, 512], F32, tag="small", bufs=3)
                        nc.tensor.matmul(
                            ps_s,
                            lhsT=kT_sb[:, h, kc * 128 : (kc + 1) * 128],
                            rhs=qT_sb[:, h, qsl],
                            start=True,
                            stop=True,
                        )
                        nc.scalar.activation(
                            pt[:, kc, :], ps_s, AF.Exp, scale=inv_sqrt_hd
                        )
                    for half in range(2):
                        ps_c = psum.tile([128, 2, HD + 1], F32, tag="ctx", bufs=2)
                        for qq in range(2):
                            for kc in range(KC):
                                nc.tensor.matmul(
                                    ps_c[:, qq, :],
                                    lhsT=pt[
                                        :,
                                        kc,
                                        (half * 2 + qq) * 128 : (half * 2 + qq + 1)
                                        * 128,
                                    ],
                                    rhs=v_sb[:, kc, h, :],
                                    start=(kc == 0),
                                    stop=(kc == KC - 1),
                                )
                        for qq in range(2):
                            qs = half * 2 + qq
                            rec = tmp_pool.tile([128, 1], F32, tag="rec")
                            nc.vector.reciprocal(rec, ps_c[:, qq, HD : HD + 1])
                            ctx_sb = tmp_pool.tile([128, 128], BF16, tag="ctxsb")
                            nc.vector.tensor_scalar_mul(
                                ctx_sb, ps_c[:, qq, 0:HD], rec
                            )
                            ps_t = psum.tile([128, 128], BF16, tag="small", bufs=3)
                            nc.tensor.transpose(ps_t, ctx_sb, ident)
                            ctxT_sb = out_pool.tile([128, 128], BF16, tag="ctxT")
                            nc.vector.tensor_copy(ctxT_sb, ps_t)
                            q0 = tok0 + qg * 512 + qs * 128
                            nc.sync.dma_start(
                                out=ctxT[h * 128 : (h + 1) * 128, q0 : q0 + 128],
                                in_=ctxT_sb,
                            )

    nc.compile()
    nc.finalize()
    return nc


def build_outproj(n_cores=N_CORES, t_slc=T_SLC, lowering=True):
    """Kernel 2: out[t, o] = ctxT_all.T @ WoT for this core's token slice."""
    nc = bacc.Bacc(
        None,
        num_devices=n_cores,
        target_bir_lowering=lowering,
        enable_partition_id=False,
    )
    ctxa = nc.dram_tensor("ctxa", [H, t_slc], BF16, kind="ExternalInput")
    woT = nc.dram_tensor("woT", [H, H], BF16, kind="ExternalInput")
    out = nc.dram_tensor("out", [t_slc, H], F32, kind="ExternalOutput")

    TT = t_slc // 128  # 4
    OB = H // 512  # 4

    with tile.TileContext(nc) as tc, ExitStack() as ctx:
        wpool = ctx.enter_context(tc.tile_pool(name="wpool", bufs=1))
        cpool = ctx.enter_context(tc.tile_pool(name="cpool", bufs=1))
        tmp = ctx.enter_context(tc.tile_pool(name="tmp", bufs=4))
        psum = ctx.enter_context(tc.tile_pool(name="psum", bufs=8, space="PSUM"))

        wo_sb = wpool.tile([128, HC, H], BF16)
        nc.sync.dma_start(out=wo_sb, in_=woT.rearrange("(c p) o -> p c o", p=128))
        ctxa_sb = cpool.tile([128, HC, t_slc], BF16)
        nc.sync.dma_start(out=ctxa_sb, in_=ctxa.rearrange("(c p) t -> p c t", p=128))

        for tt in range(TT):
            pss = [
                psum.tile([128, 512], F32, name=f"po{tt}_{ob}", tag="o")
                for ob in range(OB)
            ]
            for fc in range(HC):
                for ob in range(OB):
                    nc.tensor.matmul(
                        pss[ob],
                        lhsT=ctxa_sb[:, fc, tt * 128 : (tt + 1) * 128],
                        rhs=wo_sb[:, fc, ob * 512 : (ob + 1) * 512],
                        start=(fc == 0),
                        stop=(fc == HC - 1),
                    )
            for ob in range(OB):
                o_sb = tmp.tile([128, 512], F32, tag="osb")
                nc.vector.tensor_copy(o_sb, pss[ob])
                nc.sync.dma_start(
                    out=out[tt * 128 : (tt + 1) * 128, ob * 512 : (ob + 1) * 512],
                    in_=o_sb,
                )

    nc.compile()
    nc.finalize()
    return nc


def _prep_inputs(hidden_states, Wq, Wk, Wv, Wo, Aq, Bq, Av, Bv, position_ids):
    """Host-side layout prep: slice per core, transpose, cast, RoPE tables."""
    hs = np.ascontiguousarray(hidden_states.reshape(T, H).T).astype(np_bf16)
    woT = np.ascontiguousarray(Wo.T).astype(np_bf16)
    abT = np.ascontiguousarray(np.concatenate([Aq, Av], axis=0).T).astype(np_bf16)

    pos = np.asarray(position_ids).reshape(-1).astype(np.float64)  # [S]
    inv_freq = 1.0 / (
        ROPE_BASE ** (np.arange(0, HD, 2, dtype=np.float64) / HD)
    )  # [64]
    freqs = pos[:, None] * inv_freq[None, :]  # [S, 64]
    cos = np.cos(freqs).T.astype(np.float32)  # [64, S]
    sin = np.sin(freqs).T.astype(np.float32)
    cosT = np.concatenate([cos, cos], axis=0)  # [128, S]
    sinT = np.concatenate([-sin, sin], axis=0)  # sign-folded rotate_half

    per_core = []
    for c in range(N_CORES):
        fsl = slice(c * F_LOC, (c + 1) * F_LOC)
        per_core.append(
            dict(
                hsT=hs,
                wqT=np.ascontiguousarray(Wq[fsl, :].T).astype(np_bf16),
                wkT=np.ascontiguousarray(Wk[fsl, :].T).astype(np_bf16),
                wvT=np.ascontiguousarray(Wv[fsl, :].T).astype(np_bf16),
                abT=abT,
                bqT=np.ascontiguousarray(
                    (Bq[fsl, :] * LORA_SCALING).T
                ).astype(np_bf16),
                bvT=np.ascontiguousarray(
                    (Bv[fsl, :] * LORA_SCALING).T
                ).astype(np_bf16),
                cosT=cosT,
                sinT=sinT,
                woT=woT,
            )
        )
    return per_core


_CACHE = {}


def _get_compiled():
    if "fn" in _CACHE:
        return _CACHE["fn"]

    import jax
    from jax.sharding import Mesh, PartitionSpec as P
    from jax.experimental.shard_map import shard_map

    nc1 = build_attn()
    nc2 = build_outproj()

    attn_in = ["hsT", "wqT", "wkT", "wvT", "abT", "bqT", "bvT", "cosT", "sinT"]

    def f(hsT, wqT, wkT, wvT, abT, bqT, bvT, cosT, sinT, woT):
        (ctxT,) = bass2jax.bass_exec(
            (jax.core.ShapedArray((F_LOC, T), np_bf16),),
            tuple(attn_in),
            ("ctxT",),
            nc1,
            {},
            True,
            True,
            hsT,
            wqT,
            wkT,
            wvT,
            abT,
            bqT,
            bvT,
            cosT,
            sinT,
        )
        # exchange: [256, 8, 512] -> all cores' chunks for my token slice
        y = ctxT.reshape(F_LOC, N_CORES, T_SLC)
        g = jax.lax.all_to_all(y, "core", split_axis=1, concat_axis=0, tiled=True)
        g = g.reshape(H, T_SLC)
        (out,) = bass2jax.bass_exec(
            (jax.core.ShapedArray((T_SLC, H), np.float32),),
            ("ctxa", "woT"),
            ("out",),
            nc2,
            {},
            True,
            True,
            g,
            woT,
        )
        return out

    import jax as _jax

    mesh = Mesh(np.asarray(_jax.devices()[:N_CORES]), ("core",))
    # hsT/abT/cosT/sinT/woT replicated; w*/b* weight shards per-core
    rep = {"hsT", "abT", "cosT", "sinT", "woT"}
    names = [
        "hsT", "wqT", "wkT", "wvT", "abT", "bqT", "bvT", "cosT", "sinT", "woT",
    ]
    specs_in = tuple(P() if n in rep else P("core") for n in names)
    fn = _jax.jit(
        shard_map(
            f, mesh=mesh, in_specs=specs_in, out_specs=P("core"), check_rep=False
        )
    )
    _CACHE["fn"] = fn
    _CACHE["names"] = names
    _CACHE["rep"] = rep
    return fn


def kernel(**inputs):
    fn = _get_compiled()
    per_core = _prep_inputs(**inputs)

    names, rep = _CACHE["names"], _CACHE["rep"]
    args = [
        per_core[0][n]
        if n in rep
        else np.concatenate([per_core[c][n] for c in range(N_CORES)], axis=0)
        for n in names
    ]
    out = fn(*args)
    res = np.asarray(out)  # [N_CORES * T_SLC, H] = [T, H]
    return res.reshape(B, S, H).astype(np.float32)


# revision 13
# speedup vs baseline: 40.7782x; 40.7782x over previous
"""Trainium2 Bass kernel for nn_LlamaAttention_17085379903943.

LlamaAttention with LoRA on q/v projections + RoPE, B=2, S=2048, H=2048,
nh=16, hd=128, LoRA rank 16.

Sharding: tensor-parallel over heads across 8 NeuronCores. Each core owns 2
heads (a 256-wide slice of the qkv projection output space) and computes
q/k/v projections (+LoRA), RoPE, and full-sequence softmax attention for its
heads over both batch elements. The per-core context output ctxT
[256, 4096] (features x tokens) is exchanged with a jax-level all_to_all so
that each core ends up with all 2048 context features for a 512-token slice,
then a second bass kernel applies the output projection.  Host code only
slices/transposes/casts inputs and concatenates the 8 output slices.

Compute layout notes (PE matmul computes out = lhsT.T @ rhs, contraction on
the partition dim):
 - hs is fed transposed (hsT [H, T]) so h sits on partitions for the
   projections.  q/k are produced transposed per head (qT/kT [hd, t]) which
   is exactly the layout attention needs; v is produced natural [t, d].
 - scoresT [k, q] = kT_chunk.T @ qT, exp via ScalarE (scale=1/sqrt(hd)
   folded in) with bf16 probsT output.
 - ctx [q, d] accumulates probsT_chunk.T @ [v | ones]: the appended ones
   column makes column 128 the softmax denominator for free; eviction
   multiplies by its reciprocal (per-partition scalar broadcast).
 - RoPE is applied on the [d, t] layout: the rotate-half partition shift is
   done with two small PSUM->SBUF DMAs, sign folded into the sin table.
"""

import math
from contextlib import ExitStack

import numpy as np
import ml_dtypes

import concourse.bass as bass
import concourse.mybir as mybir
import concourse.tile as tile
from concourse import bacc
from concourse import bass2jax
from concourse.masks import make_identity

N_CORES = 8
B = 2
S = 2048
H = 2048
NH = 16
HD = 128
R = 16
LORA_SCALING = 2.0
ROPE_BASE = 10000.0

H_LOC = NH // N_CORES  # heads per core = 2
F_LOC = H_LOC * HD  # feature slice per core = 256
HC = H // 128  # h chunks = 16
T = B * S  # tokens = 4096
T_SLC = T // N_CORES  # output token slice per core = 512

BF16 = mybir.dt.bfloat16
F32 = mybir.dt.float32
np_bf16 = ml_dtypes.bfloat16

AF = mybir.ActivationFunctionType

SUMS_MODE = "dve"  # "dve": DVE partial sums + one fp32 collapse MM; "pe": ones-matmul


def build_attn(s=S, n_cores=N_CORES, lowering=True):
    """Kernel 1: qkv projections + LoRA + RoPE + attention -> ctxT."""
    t_all = B * s
    TB = s // 512  # 512-wide t blocks per batch (proj moving blocks)
    TT = s // 128  # 128-wide t tiles per batch
    QG = s // 512  # 512-wide q groups
    KC = s // 128  # k chunks

    nc = bacc.Bacc(
        None,
        num_devices=n_cores,
        target_bir_lowering=lowering,
        enable_partition_id=False,
    )

    hsT = nc.dram_tensor("hsT", [H, t_all], BF16, kind="ExternalInput")
    wqT = nc.dram_tensor("wqT", [H, F_LOC], BF16, kind="ExternalInput")
    wkT = nc.dram_tensor("wkT", [H, F_LOC], BF16, kind="ExternalInput")
    wvT = nc.dram_tensor("wvT", [H, F_LOC], BF16, kind="ExternalInput")
    abT = nc.dram_tensor("abT", [H, 2 * R], BF16, kind="ExternalInput")
    bqT = nc.dram_tensor("bqT", [R, F_LOC], BF16, kind="ExternalInput")
    bvT = nc.dram_tensor("bvT", [R, F_LOC], BF16, kind="ExternalInput")
    cosT = nc.dram_tensor("cosT", [HD, s], BF16, kind="ExternalInput")
    sinT = nc.dram_tensor("sinT", [HD, s], BF16, kind="ExternalInput")
    ctxT = nc.dram_tensor("ctxT", [F_LOC, t_all], BF16, kind="ExternalOutput")

    inv_sqrt_hd = 1.0 / math.sqrt(HD)

    with tile.TileContext(nc) as tc, ExitStack() as ctx:
        const = ctx.enter_context(tc.tile_pool(name="const", bufs=1))
        wpool = ctx.enter_context(tc.tile_pool(name="wpool", bufs=1))
        hs_pool = ctx.enter_context(tc.tile_pool(name="hs_pool", bufs=HC))
        qk_pool = ctx.enter_context(tc.tile_pool(name="qk_pool", bufs=1))
        v_pool = ctx.enter_context(tc.tile_pool(name="v_pool", bufs=1))
        lora_pool = ctx.enter_context(tc.tile_pool(name="lora_pool", bufs=1))
        probs_pool = ctx.enter_context(tc.tile_pool(name="probs_pool", bufs=2))
        tmp_pool = ctx.enter_context(tc.tile_pool(name="tmp_pool", bufs=3))
        out_pool = ctx.enter_context(tc.tile_pool(name="out_pool", bufs=4))
        psum = ctx.enter_context(tc.tile_pool(name="psum", bufs=1, space="PSUM"))

        # --- constants ---
        ident = const.tile([128, 128], BF16)
        make_identity(nc, ident)
        ones_sb = const.tile([128, 128], F32 if SUMS_MODE == "dve" else BF16)
        nc.vector.memset(ones_sb, 1.0)
        cos_sb = const.tile([HD, s], BF16)
        nc.sync.dma_start(out=cos_sb, in_=cosT[:, :])
        sin_sb = const.tile([HD, s], BF16)
        nc.sync.dma_start(out=sin_sb, in_=sinT[:, :])

        # --- weights (resident) ---
        # [h, o] views with h split into 16 chunks of 128 partitions
        wq_sb = wpool.tile([128, HC, F_LOC], BF16)
        nc.sync.dma_start(out=wq_sb, in_=wqT.rearrange("(c p) o -> p c o", p=128))
        wk_sb = wpool.tile([128, HC, F_LOC], BF16)
        nc.sync.dma_start(out=wk_sb, in_=wkT.rearrange("(c p) o -> p c o", p=128))
        wv_sb = wpool.tile([128, HC, F_LOC], BF16)
        nc.sync.dma_start(out=wv_sb, in_=wvT.rearrange("(c p) o -> p c o", p=128))
        ab_sb = wpool.tile([128, HC, 2 * R], BF16)
        nc.sync.dma_start(out=ab_sb, in_=abT.rearrange("(c p) o -> p c o", p=128))
        bq_sb = wpool.tile([R, F_LOC], BF16)
        nc.sync.dma_start(out=bq_sb, in_=bqT[:, :])
        bv_sb = wpool.tile([R, F_LOC], BF16)
        nc.sync.dma_start(out=bv_sb, in_=bvT[:, :])

        hsT_v = hsT.rearrange("(c p) t -> c p t", p=128)

        for b in range(B):
            tok0 = b * s

            # --- load hs chunks for this batch ---
            hs_tiles = []
            for c in range(HC):
                hst = hs_pool.tile([128, s], BF16, name=f"hst{c}", tag="hst")
                nc.sync.dma_start(out=hst, in_=hsT_v[c, :, tok0 : tok0 + s])
                hs_tiles.append(hst)

            # --- LoRA down-projection: [2R, s] = abT.T @ hsT ---
            lora_sb = lora_pool.tile([2 * R, s], BF16, tag="lora")
            lorav_sb = lora_pool.tile([R, s], BF16, tag="lorav")
            for tb in range(TB):
                ps_l = psum.tile([2 * R, 512], F32, tag="proj", bufs=2)
                for c in range(HC):
                    nc.tensor.matmul(
                        ps_l,
                        lhsT=ab_sb[:, c, :],
                        rhs=hs_tiles[c][:, tb * 512 : (tb + 1) * 512],
                        start=(c == 0),
                        stop=(c == HC - 1),
                    )
                nc.vector.tensor_copy(lora_sb[:, tb * 512 : (tb + 1) * 512], ps_l)
            nc.sync.dma_start(out=lorav_sb[:, :], in_=lora_sb[R : 2 * R, :])

            # --- q/k projections (+ q LoRA) with fused RoPE eviction ---
            qT_sb = qk_pool.tile([128, H_LOC, s], BF16, name="qT_sb", tag="qT")
            kT_sb = qk_pool.tile([128, H_LOC, s], BF16, name="kT_sb", tag="kT")
            for w_sb, b_lora_sb, dest in (
                (wq_sb, bq_sb, qT_sb),
                (wk_sb, None, kT_sb),
            ):
                for ot in range(H_LOC):
                    for tb in range(TB):
                        tsl = slice(tb * 512, (tb + 1) * 512)
                        ps = psum.tile([128, 512], F32, tag="proj", bufs=2)
                        for c in range(HC):
                            nc.tensor.matmul(
                                ps,
                                lhsT=w_sb[:, c, ot * 128 : (ot + 1) * 128],
                                rhs=hs_tiles[c][:, tsl],
                                start=(c == 0),
                                stop=(c == HC - 1 and b_lora_sb is None),
                            )
                        if b_lora_sb is not None:
                            nc.tensor.matmul(
                                ps,
                                lhsT=b_lora_sb[:, ot * 128 : (ot + 1) * 128],
                                rhs=lora_sb[0:R, tsl],
                                start=False,
                                stop=True,
                            )
                        # RoPE: dest = qf*cos + shift(qf)*sin'  (sign in table)
                        qf = tmp_pool.tile([128, 512], F32, tag="qf", bufs=2)
                        nc.vector.tensor_copy(qf, ps)
                        shift = tmp_pool.tile([128, 512], F32, tag="shift", bufs=2)
                        nc.sync.dma_start(out=shift[0:64, :], in_=qf[64:128, :])
                        nc.sync.dma_start(out=shift[64:128, :], in_=qf[0:64, :])
                        t1 = tmp_pool.tile([128, 512], F32, tag="t1", bufs=2)
                        nc.vector.tensor_mul(t1, shift, sin_sb[:, tsl])
                        t2 = tmp_pool.tile([128, 512], F32, tag="t2", bufs=2)
                        nc.vector.tensor_mul(t2, qf, cos_sb[:, tsl])
                        nc.vector.tensor_add(dest[:, ot, tsl], t1, t2)

            # --- v projection, transposed orientation [o, t] then PE
            #     transpose to v_sb [t, d] (ctx stationary layout) ---
            vT_sb = qk_pool.tile([128, H_LOC, s], BF16, name="vT_sb", tag="vT")
            for ot in range(H_LOC):
                for tb in range(TB):
                    tsl = slice(tb * 512, (tb + 1) * 512)
                    ps = psum.tile([128, 512], F32, tag="proj", bufs=2)
                    for c in range(HC):
                        nc.tensor.matmul(
                            ps,
                            lhsT=wv_sb[:, c, ot * 128 : (ot + 1) * 128],
                            rhs=hs_tiles[c][:, tsl],
                            start=(c == 0),
                            stop=False,
                        )
                    nc.tensor.matmul(
                        ps,
                        lhsT=bv_sb[:, ot * 128 : (ot + 1) * 128],
                        rhs=lorav_sb[0:R, tsl],
                        start=False,
                        stop=True,
                    )
                    nc.vector.tensor_copy(vT_sb[:, ot, tsl], ps)
            v_sb = v_pool.tile([128, TT, H_LOC, HD], BF16, name="v_sb", tag="v")
            for h in range(H_LOC):
                for tt in range(TT):
                    ps_t = psum.tile([128, 128], BF16, tag="small", bufs=2)
                    nc.tensor.transpose(
                        ps_t, vT_sb[:, h, tt * 128 : (tt + 1) * 128], ident
                    )
                    nc.vector.tensor_copy(v_sb[:, tt, h, :], ps_t)

            # --- attention per head: scores/probs in [k, q], ctx in [d, q] ---
            for h in range(H_LOC):
                for pair_qgs in [list(range(p, min(p + 2, QG)))
                                 for p in range(0, QG, 2)]:
                    pts = []
                    for qg in pair_qgs:
                        qsl = slice(qg * 512, (qg + 1) * 512)
                        pt = probs_pool.tile(
                            [128, KC, 512], BF16, name=f"pt{qg % 2}", tag="probs"
                        )
                        pts.append((qg, qsl, pt))
                        for kc in range(KC):
                            ps_s = psum.tile([128, 512], F32, tag="small", bufs=2)
                            nc.tensor.matmul(
                                ps_s,
                                lhsT=kT_sb[:, h, kc * 128 : (kc + 1) * 128],
                                rhs=qT_sb[:, h, qsl],
                                start=True,
                                stop=True,
                            )
                            nc.scalar.activation(
                                pt[:, kc, :], ps_s, AF.Exp, scale=inv_sqrt_hd
                            )
                    # ctx accumulation, v chunk stationary, probsT moving
                    ps_cs = {}
                    ps_bcs = {}
                    for qg, _, _ in pts:
                        ps_cs[qg] = psum.tile(
                            [128, 512], F32, name=f"ps_c{qg % 2}", tag="ctx", bufs=2
                        )
                        if SUMS_MODE == "pe":
                            ps_bcs[qg] = psum.tile(
                                [128, 512], F32, name=f"ps_b{qg % 2}",
                                tag="proj", bufs=2,
                            )
                    for kc in range(KC):
                        for qg, _, pt in pts:
                            nc.tensor.matmul(
                                ps_cs[qg],
                                lhsT=v_sb[:, kc, h, :],
                                rhs=pt[:, kc, :],
                                start=(kc == 0),
                                stop=(kc == KC - 1),
                            )
                        if SUMS_MODE == "pe":
                            for qg, _, pt in pts:
                                nc.tensor.matmul(
                                    ps_bcs[qg],
                                    lhsT=ones_sb,
                                    rhs=pt[:, kc, :],
                                    start=(kc == 0),
                                    stop=(kc == KC - 1),
                                )
                    for qg, qsl, pt in pts:
                        if SUMS_MODE == "dve":
                            # f32 partial sums over the 16 k chunks, then a
                            # single all-ones fp32 matmul collapses partitions
                            # AND broadcasts the total to every row.
                            s_acc = tmp_pool.tile([128, 512], F32, tag="sacc", bufs=2)
                            nc.vector.tensor_copy(s_acc, pt[:, 0, :])
                            for kc in range(1, KC):
                                nc.vector.tensor_add(s_acc, s_acc, pt[:, kc, :])
                            ps_bc = psum.tile(
                                [128, 512], F32, name="ps_bc", tag="proj", bufs=2
                            )
                            nc.tensor.matmul(
                                ps_bc, lhsT=ones_sb, rhs=s_acc,
                                start=True, stop=True,
                            )
                        else:
                            ps_bc = ps_bcs[qg]
                        recip = tmp_pool.tile([128, 512], F32, tag="recip", bufs=2)
                        nc.vector.reciprocal(recip, ps_bc)
                        ctxT_sb = out_pool.tile([128, 512], BF16, tag="ctxT")
                        nc.vector.tensor_mul(ctxT_sb, ps_cs[qg], recip)
                        nc.sync.dma_start(
                            out=ctxT[
                                h * 128 : (h + 1) * 128,
                                tok0 + qg * 512 : tok0 + (qg + 1) * 512,
                            ],
                            in_=ctxT_sb,
                        )

    nc.compile()
    nc.finalize()
    return nc


def build_outproj(n_cores=N_CORES, t_slc=T_SLC, lowering=True):
    """Kernel 2: out[t, o] = ctxT_all.T @ WoT for this core's token slice."""
    nc = bacc.Bacc(
        None,
        num_devices=n_cores,
        target_bir_lowering=lowering,
        enable_partition_id=False,
    )
    ctxa = nc.dram_tensor("ctxa", [H, t_slc], BF16, kind="ExternalInput")
    woT = nc.dram_tensor("woT", [H, H], BF16, kind="ExternalInput")
    out = nc.dram_tensor("out", [t_slc, H], F32, kind="ExternalOutput")

    TT = t_slc // 128  # 4
    OB = H // 512  # 4

    with tile.TileContext(nc) as tc, ExitStack() as ctx:
        wpool = ctx.enter_context(tc.tile_pool(name="wpool", bufs=2))
        cpool = ctx.enter_context(tc.tile_pool(name="cpool", bufs=1))
        tmp = ctx.enter_context(tc.tile_pool(name="tmp", bufs=4))
        psum = ctx.enter_context(tc.tile_pool(name="psum", bufs=2, space="PSUM"))

        ctxa_sb = cpool.tile([128, HC, t_slc], BF16)
        nc.sync.dma_start(out=ctxa_sb, in_=ctxa.rearrange("(c p) t -> p c t", p=128))
        woT_v = woT.rearrange("(c p) o -> p c o", p=128)

        # o-blocks outer so each WoT column block's DMA overlaps the previous
        # block's matmuls
        for ob in range(OB):
            osl = slice(ob * 512, (ob + 1) * 512)
            wo_sb = wpool.tile([128, HC, 512], BF16, name="wo_sb", tag="wo")
            nc.sync.dma_start(out=wo_sb, in_=woT_v[:, :, osl])
            for tt in range(TT):
                ps = psum.tile([128, 512], F32, tag="o", bufs=2)
                for fc in range(HC):
                    nc.tensor.matmul(
                        ps,
                        lhsT=ctxa_sb[:, fc, tt * 128 : (tt + 1) * 128],
                        rhs=wo_sb[:, fc, :],
                        start=(fc == 0),
                        stop=(fc == HC - 1),
                    )
                o_sb = tmp.tile([128, 512], F32, tag="osb")
                nc.vector.tensor_copy(o_sb, ps)
                nc.sync.dma_start(
                    out=out[tt * 128 : (tt + 1) * 128, osl],
                    in_=o_sb,
                )

    nc.compile()
    nc.finalize()
    return nc


def _prep_inputs(hidden_states, Wq, Wk, Wv, Wo, Aq, Bq, Av, Bv, position_ids):
    """Host-side layout prep: slice per core, transpose, cast, RoPE tables."""
    hs = np.ascontiguousarray(hidden_states.reshape(T, H).T).astype(np_bf16)
    woT = np.ascontiguousarray(Wo.T).astype(np_bf16)
    abT = np.ascontiguousarray(np.concatenate([Aq, Av], axis=0).T).astype(np_bf16)

    pos = np.asarray(position_ids).reshape(-1).astype(np.float64)  # [S]
    inv_freq = 1.0 / (
        ROPE_BASE ** (np.arange(0, HD, 2, dtype=np.float64) / HD)
    )  # [64]
    freqs = pos[:, None] * inv_freq[None, :]  # [S, 64]
    cos = np.cos(freqs).T.astype(np.float32)  # [64, S]
    sin = np.sin(freqs).T.astype(np.float32)
    cosT = np.concatenate([cos, cos], axis=0)  # [128, S]
    sinT = np.concatenate([-sin, sin], axis=0)  # sign-folded rotate_half

    per_core = []
    for c in range(N_CORES):
        fsl = slice(c * F_LOC, (c + 1) * F_LOC)
        per_core.append(
            dict(
                hsT=hs,
                wqT=np.ascontiguousarray(Wq[fsl, :].T).astype(np_bf16),
                wkT=np.ascontiguousarray(Wk[fsl, :].T).astype(np_bf16),
                wvT=np.ascontiguousarray(Wv[fsl, :].T).astype(np_bf16),
                abT=abT,
                bqT=np.ascontiguousarray(
                    (Bq[fsl, :] * LORA_SCALING).T
                ).astype(np_bf16),
                bvT=np.ascontiguousarray(
                    (Bv[fsl, :] * LORA_SCALING).T
                ).astype(np_bf16),
                cosT=cosT.astype(np_bf16),
                sinT=sinT.astype(np_bf16),
                woT=woT,
            )
        )
    return per_core


_CACHE = {}


def _get_compiled():
    if "fn" in _CACHE:
        return _CACHE["fn"]

    import jax
    from jax.sharding import Mesh, PartitionSpec as P
    from jax.experimental.shard_map import shard_map

    nc1 = build_attn()
    nc2 = build_outproj()

    attn_in = ["hsT", "wqT", "wkT", "wvT", "abT", "bqT", "bvT", "cosT", "sinT"]

    def f(hsT, wqT, wkT, wvT, abT, bqT, bvT, cosT, sinT, woT):
        (ctxT,) = bass2jax.bass_exec(
            (jax.core.ShapedArray((F_LOC, T), np_bf16),),
            tuple(attn_in),
            ("ctxT",),
            nc1,
            {},
            True,
            True,
            hsT,
            wqT,
            wkT,
            wvT,
            abT,
            bqT,
            bvT,
            cosT,
            sinT,
        )
        # exchange: [256, 8, 512] -> all cores' chunks for my token slice
        y = ctxT.reshape(F_LOC, N_CORES, T_SLC)
        g = jax.lax.all_to_all(y, "core", split_axis=1, concat_axis=0, tiled=True)
        g = g.reshape(H, T_SLC)
        (out,) = bass2jax.bass_exec(
            (jax.core.ShapedArray((T_SLC, H), np.float32),),
            ("ctxa", "woT"),
            ("out",),
            nc2,
            {},
            True,
            True,
            g,
            woT,
        )
        return out

    import jax as _jax

    mesh = Mesh(np.asarray(_jax.devices()[:N_CORES]), ("core",))
    # hsT/abT/cosT/sinT/woT replicated; w*/b* weight shards per-core
    rep = {"hsT", "abT", "cosT", "sinT", "woT"}
    names = [
        "hsT", "wqT", "wkT", "wvT", "abT", "bqT", "bvT", "cosT", "sinT", "woT",
    ]
    specs_in = tuple(P() if n in rep else P("core") for n in names)
    fn = _jax.jit(
        shard_map(
            f, mesh=mesh, in_specs=specs_in, out_specs=P("core"), check_rep=False
        )
    )
    _CACHE["fn"] = fn
    _CACHE["names"] = names
    _CACHE["rep"] = rep
    return fn


def kernel(**inputs):
    fn = _get_compiled()
    per_core = _prep_inputs(**inputs)

    names, rep = _CACHE["names"], _CACHE["rep"]
    args = [
        per_core[0][n]
        if n in rep
        else np.concatenate([per_core[c][n] for c in range(N_CORES)], axis=0)
        for n in names
    ]
    out = fn(*args)
    res = np.asarray(out)  # [N_CORES * T_SLC, H] = [T, H]
    return res.reshape(B, S, H).astype(np.float32)


# revision 14
# speedup vs baseline: 42.3943x; 1.0396x over previous
"""Trainium2 Bass kernel for nn_LlamaAttention_17085379903943.

LlamaAttention with LoRA on q/v projections + RoPE, B=2, S=2048, H=2048,
nh=16, hd=128, LoRA rank 16.

Sharding: tensor-parallel over heads across 8 NeuronCores. Each core owns 2
heads (a 256-wide slice of the qkv projection output space) and computes
q/k/v projections (+LoRA), RoPE, and full-sequence softmax attention for its
heads over both batch elements. The per-core context output ctxT
[256, 4096] (features x tokens) is exchanged with a jax-level all_to_all so
that each core ends up with all 2048 context features for a 512-token slice,
then a second bass kernel applies the output projection.  Host code only
slices/transposes/casts inputs and concatenates the 8 output slices.

Compute layout notes (PE matmul computes out = lhsT.T @ rhs, contraction on
the partition dim):
 - hs is fed transposed (hsT [H, T]) so h sits on partitions for the
   projections.  q/k are produced transposed per head (qT/kT [hd, t]) which
   is exactly the layout attention needs; v is produced natural [t, d].
 - scoresT [k, q] = kT_chunk.T @ qT, exp via ScalarE (scale=1/sqrt(hd)
   folded in) with bf16 probsT output.
 - ctx [q, d] accumulates probsT_chunk.T @ [v | ones]: the appended ones
   column makes column 128 the softmax denominator for free; eviction
   multiplies by its reciprocal (per-partition scalar broadcast).
 - RoPE is applied on the [d, t] layout: the rotate-half partition shift is
   done with two small PSUM->SBUF DMAs, sign folded into the sin table.
"""

import math
from contextlib import ExitStack

import numpy as np
import ml_dtypes

import concourse.bass as bass
import concourse.mybir as mybir
import concourse.tile as tile
from concourse import bacc
from concourse import bass2jax
from concourse.masks import make_identity

N_CORES = 8
B = 2
S = 2048
H = 2048
NH = 16
HD = 128
R = 16
LORA_SCALING = 2.0
ROPE_BASE = 10000.0

H_LOC = NH // N_CORES  # heads per core = 2
F_LOC = H_LOC * HD  # feature slice per core = 256
HC = H // 128  # h chunks = 16
T = B * S  # tokens = 4096
T_SLC = T // N_CORES  # output token slice per core = 512

BF16 = mybir.dt.bfloat16
F32 = mybir.dt.float32
np_bf16 = ml_dtypes.bfloat16

AF = mybir.ActivationFunctionType

SUMS_MODE = "pe"  # "dve": DVE partial sums + one fp32 collapse MM; "pe": ones-matmul


def build_attn(s=S, n_cores=N_CORES, lowering=True):
    """Kernel 1: qkv projections + LoRA + RoPE + attention -> ctxT."""
    t_all = B * s
    TB = s // 512  # 512-wide t blocks per batch (proj moving blocks)
    TT = s // 128  # 128-wide t tiles per batch
    QG = s // 512  # 512-wide q groups
    KC = s // 128  # k chunks

    nc = bacc.Bacc(
        None,
        num_devices=n_cores,
        target_bir_lowering=lowering,
        enable_partition_id=False,
    )

    hsT = nc.dram_tensor("hsT", [H, t_all], BF16, kind="ExternalInput")
    wqT = nc.dram_tensor("wqT", [H, F_LOC], BF16, kind="ExternalInput")
    wkT = nc.dram_tensor("wkT", [H, F_LOC], BF16, kind="ExternalInput")
    wvT = nc.dram_tensor("wvT", [H, F_LOC], BF16, kind="ExternalInput")
    abT = nc.dram_tensor("abT", [H, 2 * R], BF16, kind="ExternalInput")
    bqT = nc.dram_tensor("bqT", [R, F_LOC], BF16, kind="ExternalInput")
    bvT = nc.dram_tensor("bvT", [R, F_LOC], BF16, kind="ExternalInput")
    cosT = nc.dram_tensor("cosT", [HD, s], BF16, kind="ExternalInput")
    sinT = nc.dram_tensor("sinT", [HD, s], BF16, kind="ExternalInput")
    ctxT = nc.dram_tensor("ctxT", [F_LOC, t_all], BF16, kind="ExternalOutput")

    inv_sqrt_hd = 1.0 / math.sqrt(HD)

    with tile.TileContext(nc) as tc, ExitStack() as ctx:
        const = ctx.enter_context(tc.tile_pool(name="const", bufs=1))
        wpool = ctx.enter_context(tc.tile_pool(name="wpool", bufs=1))
        hs_pool = ctx.enter_context(tc.tile_pool(name="hs_pool", bufs=HC))
        qk_pool = ctx.enter_context(tc.tile_pool(name="qk_pool", bufs=1))
        v_pool = ctx.enter_context(tc.tile_pool(name="v_pool", bufs=1))
        lora_pool = ctx.enter_context(tc.tile_pool(name="lora_pool", bufs=1))
        probs_pool = ctx.enter_context(tc.tile_pool(name="probs_pool", bufs=2))
        tmp_pool = ctx.enter_context(tc.tile_pool(name="tmp_pool", bufs=3))
        out_pool = ctx.enter_context(tc.tile_pool(name="out_pool", bufs=4))
        psum = ctx.enter_context(tc.tile_pool(name="psum", bufs=1, space="PSUM"))

        # --- constants ---
        ident = const.tile([128, 128], BF16)
        make_identity(nc, ident)
        ones_sb = const.tile([128, 128], F32 if SUMS_MODE == "dve" else BF16)
        nc.vector.memset(ones_sb, 1.0)
        cos_sb = const.tile([HD, s], BF16)
        nc.sync.dma_start(out=cos_sb, in_=cosT[:, :])
        sin_sb = const.tile([HD, s], BF16)
        nc.sync.dma_start(out=sin_sb, in_=sinT[:, :])

        # --- weights (resident) ---
        # [h, o] views with h split into 16 chunks of 128 partitions
        wq_sb = wpool.tile([128, HC, F_LOC], BF16)
        nc.sync.dma_start(out=wq_sb, in_=wqT.rearrange("(c p) o -> p c o", p=128))
        wk_sb = wpool.tile([128, HC, F_LOC], BF16)
        nc.sync.dma_start(out=wk_sb, in_=wkT.rearrange("(c p) o -> p c o", p=128))
        wv_sb = wpool.tile([128, HC, F_LOC], BF16)
        nc.sync.dma_start(out=wv_sb, in_=wvT.rearrange("(c p) o -> p c o", p=128))
        ab_sb = wpool.tile([128, HC, 2 * R], BF16)
        nc.sync.dma_start(out=ab_sb, in_=abT.rearrange("(c p) o -> p c o", p=128))
        bq_sb = wpool.tile([R, F_LOC], BF16)
        nc.sync.dma_start(out=bq_sb, in_=bqT[:, :])
        bv_sb = wpool.tile([R, F_LOC], BF16)
        nc.sync.dma_start(out=bv_sb, in_=bvT[:, :])

        hsT_v = hsT.rearrange("(c p) t -> c p t", p=128)

        for b in range(B):
            tok0 = b * s

            # --- load hs chunks for this batch ---
            hs_tiles = []
            for c in range(HC):
                hst = hs_pool.tile([128, s], BF16, name=f"hst{c}", tag="hst")
                nc.sync.dma_start(out=hst, in_=hsT_v[c, :, tok0 : tok0 + s])
                hs_tiles.append(hst)

            # --- LoRA down-projection: [2R, s] = abT.T @ hsT ---
            lora_sb = lora_pool.tile([2 * R, s], BF16, tag="lora")
            lorav_sb = lora_pool.tile([R, s], BF16, tag="lorav")
            for tb in range(TB):
                ps_l = psum.tile([2 * R, 512], F32, tag="proj", bufs=2)
                for c in range(HC):
                    nc.tensor.matmul(
                        ps_l,
                        lhsT=ab_sb[:, c, :],
                        rhs=hs_tiles[c][:, tb * 512 : (tb + 1) * 512],
                        start=(c == 0),
                        stop=(c == HC - 1),
                    )
                nc.vector.tensor_copy(lora_sb[:, tb * 512 : (tb + 1) * 512], ps_l)
            nc.sync.dma_start(out=lorav_sb[:, :], in_=lora_sb[R : 2 * R, :])

            # --- q/k projections (+ q LoRA) with fused RoPE eviction ---
            qT_sb = qk_pool.tile([128, H_LOC, s], BF16, name="qT_sb", tag="qT")
            kT_sb = qk_pool.tile([128, H_LOC, s], BF16, name="kT_sb", tag="kT")
            for w_sb, b_lora_sb, dest in (
                (wq_sb, bq_sb, qT_sb),
                (wk_sb, None, kT_sb),
            ):
                for ot in range(H_LOC):
                    for tb in range(TB):
                        tsl = slice(tb * 512, (tb + 1) * 512)
                        ps = psum.tile([128, 512], F32, tag="proj", bufs=2)
                        for c in range(HC):
                            nc.tensor.matmul(
                                ps,
                                lhsT=w_sb[:, c, ot * 128 : (ot + 1) * 128],
                                rhs=hs_tiles[c][:, tsl],
                                start=(c == 0),
                                stop=(c == HC - 1 and b_lora_sb is None),
                            )
                        if b_lora_sb is not None:
                            nc.tensor.matmul(
                                ps,
                                lhsT=b_lora_sb[:, ot * 128 : (ot + 1) * 128],
                                rhs=lora_sb[0:R, tsl],
                                start=False,
                                stop=True,
                            )
                        # RoPE: dest = qf*cos + shift(qf)*sin'  (sign in table)
                        qf = tmp_pool.tile([128, 512], F32, tag="qf", bufs=2)
                        nc.vector.tensor_copy(qf, ps)
                        shift = tmp_pool.tile([128, 512], F32, tag="shift", bufs=2)
                        nc.sync.dma_start(out=shift[0:64, :], in_=qf[64:128, :])
                        nc.sync.dma_start(out=shift[64:128, :], in_=qf[0:64, :])
                        t1 = tmp_pool.tile([128, 512], F32, tag="t1", bufs=2)
                        nc.vector.tensor_mul(t1, shift, sin_sb[:, tsl])
                        t2 = tmp_pool.tile([128, 512], F32, tag="t2", bufs=2)
                        nc.vector.tensor_mul(t2, qf, cos_sb[:, tsl])
                        nc.vector.tensor_add(dest[:, ot, tsl], t1, t2)

            # --- v projection, transposed orientation [o, t] then PE
            #     transpose to v_sb [t, d] (ctx stationary layout) ---
            vT_sb = qk_pool.tile([128, H_LOC, s], BF16, name="vT_sb", tag="vT")
            for ot in range(H_LOC):
                for tb in range(TB):
                    tsl = slice(tb * 512, (tb + 1) * 512)
                    ps = psum.tile([128, 512], F32, tag="proj", bufs=2)
                    for c in range(HC):
                        nc.tensor.matmul(
                            ps,
                            lhsT=wv_sb[:, c, ot * 128 : (ot + 1) * 128],
                            rhs=hs_tiles[c][:, tsl],
                            start=(c == 0),
                            stop=False,
                        )
                    nc.tensor.matmul(
                        ps,
                        lhsT=bv_sb[:, ot * 128 : (ot + 1) * 128],
                        rhs=lorav_sb[0:R, tsl],
                        start=False,
                        stop=True,
                    )
                    nc.vector.tensor_copy(vT_sb[:, ot, tsl], ps)
            v_sb = v_pool.tile([128, TT, H_LOC, HD], BF16, name="v_sb", tag="v")
            for h in range(H_LOC):
                for tt in range(TT):
                    ps_t = psum.tile([128, 128], BF16, tag="small", bufs=2)
                    nc.tensor.transpose(
                        ps_t, vT_sb[:, h, tt * 128 : (tt + 1) * 128], ident
                    )
                    nc.vector.tensor_copy(v_sb[:, tt, h, :], ps_t)

            # --- attention per head: scores/probs in [k, q], ctx in [d, q] ---
            for h in range(H_LOC):
                for pair_qgs in [list(range(p, min(p + 2, QG)))
                                 for p in range(0, QG, 2)]:
                    pts = []
                    for qg in pair_qgs:
                        qsl = slice(qg * 512, (qg + 1) * 512)
                        pt = probs_pool.tile(
                            [128, KC, 512], BF16, name=f"pt{qg % 2}", tag="probs"
                        )
                        pts.append((qg, qsl, pt))
                        for kc in range(KC):
                            ps_s = psum.tile([128, 512], F32, tag="small", bufs=2)
                            nc.tensor.matmul(
                                ps_s,
                                lhsT=kT_sb[:, h, kc * 128 : (kc + 1) * 128],
                                rhs=qT_sb[:, h, qsl],
                                start=True,
                                stop=True,
                            )
                            nc.scalar.activation(
                                pt[:, kc, :], ps_s, AF.Exp, scale=inv_sqrt_hd
                            )
                    # ctx accumulation, v chunk stationary, probsT moving
                    ps_cs = {}
                    ps_bcs = {}
                    for qg, _, _ in pts:
                        ps_cs[qg] = psum.tile(
                            [128, 512], F32, name=f"ps_c{qg % 2}", tag="ctx", bufs=2
                        )
                        if SUMS_MODE == "pe":
                            ps_bcs[qg] = psum.tile(
                                [128, 512], F32, name=f"ps_b{qg % 2}",
                                tag="proj", bufs=2,
                            )
                    for kc in range(KC):
                        for qg, _, pt in pts:
                            nc.tensor.matmul(
                                ps_cs[qg],
                                lhsT=v_sb[:, kc, h, :],
                                rhs=pt[:, kc, :],
                                start=(kc == 0),
                                stop=(kc == KC - 1),
                            )
                        if SUMS_MODE == "pe":
                            for qg, _, pt in pts:
                                nc.tensor.matmul(
                                    ps_bcs[qg],
                                    lhsT=ones_sb,
                                    rhs=pt[:, kc, :],
                                    start=(kc == 0),
                                    stop=(kc == KC - 1),
                                )
                    for qg, qsl, pt in pts:
                        if SUMS_MODE == "dve":
                            # f32 partial sums over the 16 k chunks, then a
                            # single all-ones fp32 matmul collapses partitions
                            # AND broadcasts the total to every row.
                            s_acc = tmp_pool.tile([128, 512], F32, tag="sacc", bufs=2)
                            nc.vector.tensor_copy(s_acc, pt[:, 0, :])
                            for kc in range(1, KC):
                                nc.vector.tensor_add(s_acc, s_acc, pt[:, kc, :])
                            ps_bc = psum.tile(
                                [128, 512], F32, name="ps_bc", tag="proj", bufs=2
                            )
                            nc.tensor.matmul(
                                ps_bc, lhsT=ones_sb, rhs=s_acc,
                                start=True, stop=True,
                            )
                        else:
                            ps_bc = ps_bcs[qg]
                        recip = tmp_pool.tile([128, 512], F32, tag="recip", bufs=2)
                        nc.vector.reciprocal_approx_fast(out=recip, in_=ps_bc)
                        ctxT_sb = out_pool.tile([128, 512], BF16, tag="ctxT")
                        nc.vector.tensor_mul(ctxT_sb, ps_cs[qg], recip)
                        nc.sync.dma_start(
                            out=ctxT[
                                h * 128 : (h + 1) * 128,
                                tok0 + qg * 512 : tok0 + (qg + 1) * 512,
                            ],
                            in_=ctxT_sb,
                        )

    nc.compile()
    nc.finalize()
    return nc


def build_outproj(n_cores=N_CORES, t_slc=T_SLC, lowering=True):
    """Kernel 2: out[t, o] = ctxT_all.T @ WoT for this core's token slice."""
    nc = bacc.Bacc(
        None,
        num_devices=n_cores,
        target_bir_lowering=lowering,
        enable_partition_id=False,
    )
    ctxa = nc.dram_tensor("ctxa", [H, t_slc], BF16, kind="ExternalInput")
    woT = nc.dram_tensor("woT", [H, H], BF16, kind="ExternalInput")
    out = nc.dram_tensor("out", [t_slc, H], F32, kind="ExternalOutput")

    TT = t_slc // 128  # 4
    OB = H // 512  # 4

    with tile.TileContext(nc) as tc, ExitStack() as ctx:
        wpool = ctx.enter_context(tc.tile_pool(name="wpool", bufs=2))
        cpool = ctx.enter_context(tc.tile_pool(name="cpool", bufs=1))
        tmp = ctx.enter_context(tc.tile_pool(name="tmp", bufs=4))
        psum = ctx.enter_context(tc.tile_pool(name="psum", bufs=2, space="PSUM"))

        ctxa_sb = cpool.tile([128, HC, t_slc], BF16)
        nc.sync.dma_start(out=ctxa_sb, in_=ctxa.rearrange("(c p) t -> p c t", p=128))
        woT_v = woT.rearrange("(c p) o -> p c o", p=128)

        # o-blocks outer so each WoT column block's DMA overlaps the previous
        # block's matmuls
        for ob in range(OB):
            osl = slice(ob * 512, (ob + 1) * 512)
            wo_sb = wpool.tile([128, HC, 512], BF16, name="wo_sb", tag="wo")
            nc.sync.dma_start(out=wo_sb, in_=woT_v[:, :, osl])
            for tt in range(TT):
                ps = psum.tile([128, 512], F32, tag="o", bufs=2)
                for fc in range(HC):
                    nc.tensor.matmul(
                        ps,
                        lhsT=ctxa_sb[:, fc, tt * 128 : (tt + 1) * 128],
                        rhs=wo_sb[:, fc, :],
                        start=(fc == 0),
                        stop=(fc == HC - 1),
                    )
                o_sb = tmp.tile([128, 512], F32, tag="osb")
                nc.vector.tensor_copy(o_sb, ps)
                nc.sync.dma_start(
                    out=out[tt * 128 : (tt + 1) * 128, osl],
                    in_=o_sb,
                )

    nc.compile()
    nc.finalize()
    return nc


def _prep_inputs(hidden_states, Wq, Wk, Wv, Wo, Aq, Bq, Av, Bv, position_ids):
    """Host-side layout prep: slice per core, transpose, cast, RoPE tables."""
    hs = np.ascontiguousarray(hidden_states.reshape(T, H).T).astype(np_bf16)
    woT = np.ascontiguousarray(Wo.T).astype(np_bf16)
    abT = np.ascontiguousarray(np.concatenate([Aq, Av], axis=0).T).astype(np_bf16)

    pos = np.asarray(position_ids).reshape(-1).astype(np.float64)  # [S]
    inv_freq = 1.0 / (
        ROPE_BASE ** (np.arange(0, HD, 2, dtype=np.float64) / HD)
    )  # [64]
    freqs = pos[:, None] * inv_freq[None, :]  # [S, 64]
    cos = np.cos(freqs).T.astype(np.float32)  # [64, S]
    sin = np.sin(freqs).T.astype(np.float32)
    cosT = np.concatenate([cos, cos], axis=0)  # [128, S]
    sinT = np.concatenate([-sin, sin], axis=0)  # sign-folded rotate_half

    per_core = []
    for c in range(N_CORES):
        fsl = slice(c * F_LOC, (c + 1) * F_LOC)
        per_core.append(
            dict(
                hsT=hs,
                wqT=np.ascontiguousarray(Wq[fsl, :].T).astype(np_bf16),
                wkT=np.ascontiguousarray(Wk[fsl, :].T).astype(np_bf16),
                wvT=np.ascontiguousarray(Wv[fsl, :].T).astype(np_bf16),
                abT=abT,
                bqT=np.ascontiguousarray(
                    (Bq[fsl, :] * LORA_SCALING).T
                ).astype(np_bf16),
                bvT=np.ascontiguousarray(
                    (Bv[fsl, :] * LORA_SCALING).T
                ).astype(np_bf16),
                cosT=cosT.astype(np_bf16),
                sinT=sinT.astype(np_bf16),
                woT=woT,
            )
        )
    return per_core


_CACHE = {}


def _get_compiled():
    if "fn" in _CACHE:
        return _CACHE["fn"]

    import jax
    from jax.sharding import Mesh, PartitionSpec as P
    from jax.experimental.shard_map import shard_map

    nc1 = build_attn()
    nc2 = build_outproj()

    attn_in = ["hsT", "wqT", "wkT", "wvT", "abT", "bqT", "bvT", "cosT", "sinT"]

    def f(hsT, wqT, wkT, wvT, abT, bqT, bvT, cosT, sinT, woT):
        (ctxT,) = bass2jax.bass_exec(
            (jax.core.ShapedArray((F_LOC, T), np_bf16),),
            tuple(attn_in),
            ("ctxT",),
            nc1,
            {},
            True,
            True,
            hsT,
            wqT,
            wkT,
            wvT,
            abT,
            bqT,
            bvT,
            cosT,
            sinT,
        )
        # exchange: [256, 8, 512] -> all cores' chunks for my token slice
        y = ctxT.reshape(F_LOC, N_CORES, T_SLC)
        g = jax.lax.all_to_all(y, "core", split_axis=1, concat_axis=0, tiled=True)
        g = g.reshape(H, T_SLC)
        (out,) = bass2jax.bass_exec(
            (jax.core.ShapedArray((T_SLC, H), np.float32),),
            ("ctxa", "woT"),
            ("out",),
            nc2,
            {},
            True,
            True,
            g,
            woT,
        )
        return out

    import jax as _jax

    mesh = Mesh(np.asarray(_jax.devices()[:N_CORES]), ("core",))
    # hsT/abT/cosT/sinT/woT replicated; w*/b* weight shards per-core
    rep = {"hsT", "abT", "cosT", "sinT", "woT"}
    names = [
        "hsT", "wqT", "wkT", "wvT", "abT", "bqT", "bvT", "cosT", "sinT", "woT",
    ]
    specs_in = tuple(P() if n in rep else P("core") for n in names)
    fn = _jax.jit(
        shard_map(
            f, mesh=mesh, in_specs=specs_in, out_specs=P("core"), check_rep=False
        )
    )
    _CACHE["fn"] = fn
    _CACHE["names"] = names
    _CACHE["rep"] = rep
    return fn


def kernel(**inputs):
    fn = _get_compiled()
    per_core = _prep_inputs(**inputs)

    names, rep = _CACHE["names"], _CACHE["rep"]
    args = [
        per_core[0][n]
        if n in rep
        else np.concatenate([per_core[c][n] for c in range(N_CORES)], axis=0)
        for n in names
    ]
    out = fn(*args)
    res = np.asarray(out)  # [N_CORES * T_SLC, H] = [T, H]
    return res.reshape(B, S, H).astype(np.float32)


# revision 18
# speedup vs baseline: 44.9288x; 1.0598x over previous
"""Trainium2 Bass kernel for nn_LlamaAttention_17085379903943.

LlamaAttention with LoRA on q/v projections + RoPE, B=2, S=2048, H=2048,
nh=16, hd=128, LoRA rank 16.

Sharding: tensor-parallel over heads across 8 NeuronCores. Each core owns 2
heads (a 256-wide slice of the qkv projection output space) and computes
q/k/v projections (+LoRA), RoPE, and full-sequence softmax attention for its
heads over both batch elements. The per-core context output ctxT
[256, 4096] (features x tokens) is exchanged with a jax-level all_to_all so
that each core ends up with all 2048 context features for a 512-token slice,
then a second bass kernel applies the output projection.  Host code only
slices/transposes/casts inputs and concatenates the 8 output slices.

Compute layout notes (PE matmul computes out = lhsT.T @ rhs, contraction on
the partition dim):
 - hs is fed transposed (hsT [H, T]) so h sits on partitions for the
   projections.  q/k are produced transposed per head (qT/kT [hd, t]) which
   is exactly the layout attention needs; v is produced natural [t, d].
 - scoresT [k, q] = kT_chunk.T @ qT, exp via ScalarE (scale=1/sqrt(hd)
   folded in) with bf16 probsT output.
 - ctx [q, d] accumulates probsT_chunk.T @ [v | ones]: the appended ones
   column makes column 128 the softmax denominator for free; eviction
   multiplies by its reciprocal (per-partition scalar broadcast).
 - RoPE is applied on the [d, t] layout: the rotate-half partition shift is
   done with two small PSUM->SBUF DMAs, sign folded into the sin table.
"""

import math
from contextlib import ExitStack

import numpy as np
import ml_dtypes

import concourse.bass as bass
import concourse.mybir as mybir
import concourse.tile as tile
from concourse import bacc
from concourse import bass2jax
from concourse.masks import make_identity

N_CORES = 8
B = 2
S = 2048
H = 2048
NH = 16
HD = 128
R = 16
LORA_SCALING = 2.0
ROPE_BASE = 10000.0

H_LOC = NH // N_CORES  # heads per core = 2
F_LOC = H_LOC * HD  # feature slice per core = 256
HC = H // 128  # h chunks = 16
T = B * S  # tokens = 4096
T_SLC = T // N_CORES  # output token slice per core = 512

BF16 = mybir.dt.bfloat16
F32 = mybir.dt.float32
np_bf16 = ml_dtypes.bfloat16

AF = mybir.ActivationFunctionType

SUMS_MODE = "pe"  # "dve": DVE partial sums + one fp32 collapse MM; "pe": ones-matmul


def build_attn(s=S, n_cores=N_CORES, lowering=True):
    """Kernel 1: qkv projections + LoRA + RoPE + attention -> ctxT."""
    t_all = B * s
    TB = s // 512  # 512-wide t blocks per batch (proj moving blocks)
    TT = s // 128  # 128-wide t tiles per batch
    QG = s // 512  # 512-wide q groups
    KC = s // 128  # k chunks

    nc = bacc.Bacc(
        None,
        num_devices=n_cores,
        target_bir_lowering=lowering,
        enable_partition_id=False,
    )

    hsT = nc.dram_tensor("hsT", [H, t_all], BF16, kind="ExternalInput")
    wqT = nc.dram_tensor("wqT", [H, F_LOC], BF16, kind="ExternalInput")
    wkT = nc.dram_tensor("wkT", [H, F_LOC], BF16, kind="ExternalInput")
    wvT = nc.dram_tensor("wvT", [H, F_LOC], BF16, kind="ExternalInput")
    abT = nc.dram_tensor("abT", [H, 2 * R], BF16, kind="ExternalInput")
    bqT = nc.dram_tensor("bqT", [R, F_LOC], BF16, kind="ExternalInput")
    bvT = nc.dram_tensor("bvT", [R, F_LOC], BF16, kind="ExternalInput")
    cosT = nc.dram_tensor("cosT", [HD, s], BF16, kind="ExternalInput")
    sinT = nc.dram_tensor("sinT", [HD, s], BF16, kind="ExternalInput")
    n_slc = t_all // n_cores
    ctxT = nc.dram_tensor("ctxT", [n_cores, F_LOC, n_slc], BF16, kind="ExternalOutput")

    inv_sqrt_hd = 1.0 / math.sqrt(HD)

    with tile.TileContext(nc) as tc, ExitStack() as ctx:
        const = ctx.enter_context(tc.tile_pool(name="const", bufs=1))
        wpool = ctx.enter_context(tc.tile_pool(name="wpool", bufs=1))
        hs_pool = ctx.enter_context(tc.tile_pool(name="hs_pool", bufs=HC))
        qk_pool = ctx.enter_context(tc.tile_pool(name="qk_pool", bufs=1))
        v_pool = ctx.enter_context(tc.tile_pool(name="v_pool", bufs=1))
        lora_pool = ctx.enter_context(tc.tile_pool(name="lora_pool", bufs=1))
        probs_pool = ctx.enter_context(tc.tile_pool(name="probs_pool", bufs=2))
        tmp_pool = ctx.enter_context(tc.tile_pool(name="tmp_pool", bufs=3))
        out_pool = ctx.enter_context(tc.tile_pool(name="out_pool", bufs=4))
        psum = ctx.enter_context(tc.tile_pool(name="psum", bufs=1, space="PSUM"))

        # --- constants ---
        ident = const.tile([128, 128], BF16)
        make_identity(nc, ident)
        ones_sb = const.tile([128, 128], F32 if SUMS_MODE == "dve" else BF16)
        nc.vector.memset(ones_sb, 1.0)
        cos_sb = const.tile([HD, s], BF16)
        nc.scalar.dma_start(out=cos_sb, in_=cosT[:, :])
        sin_sb = const.tile([HD, s], BF16)
        nc.scalar.dma_start(out=sin_sb, in_=sinT[:, :])

        # --- weights (resident) ---
        # [h, o] views with h split into 16 chunks of 128 partitions
        wq_sb = wpool.tile([128, HC, F_LOC], BF16)
        nc.scalar.dma_start(out=wq_sb, in_=wqT.rearrange("(c p) o -> p c o", p=128))
        wk_sb = wpool.tile([128, HC, F_LOC], BF16)
        nc.scalar.dma_start(out=wk_sb, in_=wkT.rearrange("(c p) o -> p c o", p=128))
        wv_sb = wpool.tile([128, HC, F_LOC], BF16)
        nc.scalar.dma_start(out=wv_sb, in_=wvT.rearrange("(c p) o -> p c o", p=128))
        ab_sb = wpool.tile([128, HC, 2 * R], BF16)
        nc.scalar.dma_start(out=ab_sb, in_=abT.rearrange("(c p) o -> p c o", p=128))
        bq_sb = wpool.tile([R, F_LOC], BF16)
        nc.scalar.dma_start(out=bq_sb, in_=bqT[:, :])
        bv_sb = wpool.tile([R, F_LOC], BF16)
        nc.scalar.dma_start(out=bv_sb, in_=bvT[:, :])

        hsT_v = hsT.rearrange("(c p) t -> c p t", p=128)

        for b in range(B):
            tok0 = b * s

            # --- load hs chunks for this batch ---
            hs_tiles = []
            for c in range(HC):
                hst = hs_pool.tile([128, s], BF16, name=f"hst{c}", tag="hst")
                nc.sync.dma_start(out=hst, in_=hsT_v[c, :, tok0 : tok0 + s])
                hs_tiles.append(hst)

            # --- LoRA down-projection: [2R, s] = abT.T @ hsT ---
            lora_sb = lora_pool.tile([2 * R, s], BF16, tag="lora")
            lorav_sb = lora_pool.tile([R, s], BF16, tag="lorav")
            for tb in range(TB):
                ps_l = psum.tile([2 * R, 512], F32, tag="proj", bufs=2)
                for c in range(HC):
                    nc.tensor.matmul(
                        ps_l,
                        lhsT=ab_sb[:, c, :],
                        rhs=hs_tiles[c][:, tb * 512 : (tb + 1) * 512],
                        start=(c == 0),
                        stop=(c == HC - 1),
                    )
                nc.vector.tensor_copy(lora_sb[:, tb * 512 : (tb + 1) * 512], ps_l)
            nc.sync.dma_start(out=lorav_sb[:, :], in_=lora_sb[R : 2 * R, :])

            # --- q/k projections (+ q LoRA) with fused RoPE eviction ---
            qT_sb = qk_pool.tile([128, H_LOC, s], BF16, name="qT_sb", tag="qT")
            kT_sb = qk_pool.tile([128, H_LOC, s], BF16, name="kT_sb", tag="kT")
            for w_sb, b_lora_sb, dest in (
                (wq_sb, bq_sb, qT_sb),
                (wk_sb, None, kT_sb),
            ):
                for ot in range(H_LOC):
                    for tb in range(TB):
                        tsl = slice(tb * 512, (tb + 1) * 512)
                        ps = psum.tile([128, 512], F32, tag="proj", bufs=2)
                        for c in range(HC):
                            nc.tensor.matmul(
                                ps,
                                lhsT=w_sb[:, c, ot * 128 : (ot + 1) * 128],
                                rhs=hs_tiles[c][:, tsl],
                                start=(c == 0),
                                stop=(c == HC - 1 and b_lora_sb is None),
                            )
                        if b_lora_sb is not None:
                            nc.tensor.matmul(
                                ps,
                                lhsT=b_lora_sb[:, ot * 128 : (ot + 1) * 128],
                                rhs=lora_sb[0:R, tsl],
                                start=False,
                                stop=True,
                            )
                        # RoPE: dest = qf*cos + shift(qf)*sin'  (sign in table)
                        qf = tmp_pool.tile([128, 512], F32, tag="qf", bufs=2)
                        nc.vector.tensor_copy(qf, ps)
                        shift = tmp_pool.tile([128, 512], F32, tag="shift", bufs=2)
                        nc.sync.dma_start(out=shift[0:64, :], in_=qf[64:128, :])
                        nc.sync.dma_start(out=shift[64:128, :], in_=qf[0:64, :])
                        t1 = tmp_pool.tile([128, 512], F32, tag="t1", bufs=2)
                        nc.vector.tensor_mul(t1, shift, sin_sb[:, tsl])
                        t2 = tmp_pool.tile([128, 512], F32, tag="t2", bufs=2)
                        nc.vector.tensor_mul(t2, qf, cos_sb[:, tsl])
                        nc.vector.tensor_add(dest[:, ot, tsl], t1, t2)

            # --- v projection, transposed orientation [o, t] then PE
            #     transpose to v_sb [t, d] (ctx stationary layout) ---
            vT_sb = qk_pool.tile([128, H_LOC, s], BF16, name="vT_sb", tag="vT")
            for ot in range(H_LOC):
                for tb in range(TB):
                    tsl = slice(tb * 512, (tb + 1) * 512)
                    ps = psum.tile([128, 512], F32, tag="proj", bufs=2)
                    for c in range(HC):
                        nc.tensor.matmul(
                            ps,
                            lhsT=wv_sb[:, c, ot * 128 : (ot + 1) * 128],
                            rhs=hs_tiles[c][:, tsl],
                            start=(c == 0),
                            stop=False,
                        )
                    nc.tensor.matmul(
                        ps,
                        lhsT=bv_sb[:, ot * 128 : (ot + 1) * 128],
                        rhs=lorav_sb[0:R, tsl],
                        start=False,
                        stop=True,
                    )
                    nc.vector.tensor_copy(vT_sb[:, ot, tsl], ps)
            v_sb = v_pool.tile([128, TT, H_LOC, HD], BF16, name="v_sb", tag="v")
            for h in range(H_LOC):
                for tt in range(TT):
                    ps_t = psum.tile([128, 128], BF16, tag="small", bufs=2)
                    nc.tensor.transpose(
                        ps_t, vT_sb[:, h, tt * 128 : (tt + 1) * 128], ident
                    )
                    nc.vector.tensor_copy(v_sb[:, tt, h, :], ps_t)

            # --- attention per head: scores/probs in [k, q], ctx in [d, q] ---
            for h in range(H_LOC):
                for pair_qgs in [list(range(p, min(p + 2, QG)))
                                 for p in range(0, QG, 2)]:
                    pts = []
                    for qg in pair_qgs:
                        qsl = slice(qg * 512, (qg + 1) * 512)
                        pt = probs_pool.tile(
                            [128, KC, 512], BF16, name=f"pt{qg % 2}", tag="probs"
                        )
                        pts.append((qg, qsl, pt))
                        for kc in range(KC):
                            ps_s = psum.tile([128, 512], F32, tag="small", bufs=2)
                            nc.tensor.matmul(
                                ps_s,
                                lhsT=kT_sb[:, h, kc * 128 : (kc + 1) * 128],
                                rhs=qT_sb[:, h, qsl],
                                start=True,
                                stop=True,
                            )
                            nc.scalar.activation(
                                pt[:, kc, :], ps_s, AF.Exp, scale=inv_sqrt_hd
                            )
                    # ctx accumulation, v chunk stationary, probsT moving
                    ps_cs = {}
                    ps_bcs = {}
                    for qg, _, _ in pts:
                        ps_cs[qg] = psum.tile(
                            [128, 512], F32, name=f"ps_c{qg % 2}", tag="ctx", bufs=2
                        )
                        if SUMS_MODE == "pe":
                            ps_bcs[qg] = psum.tile(
                                [128, 512], F32, name=f"ps_b{qg % 2}",
                                tag="proj", bufs=2,
                            )
                    for kc in range(KC):
                        for qg, _, pt in pts:
                            nc.tensor.matmul(
                                ps_cs[qg],
                                lhsT=v_sb[:, kc, h, :],
                                rhs=pt[:, kc, :],
                                start=(kc == 0),
                                stop=(kc == KC - 1),
                            )
                        if SUMS_MODE == "pe":
                            for qg, _, pt in pts:
                                nc.tensor.matmul(
                                    ps_bcs[qg],
                                    lhsT=ones_sb,
                                    rhs=pt[:, kc, :],
                                    start=(kc == 0),
                                    stop=(kc == KC - 1),
                                )
                    for qg, qsl, pt in pts:
                        if SUMS_MODE == "dve":
                            # f32 partial sums over the 16 k chunks, then a
                            # single all-ones fp32 matmul collapses partitions
                            # AND broadcasts the total to every row.
                            s_acc = tmp_pool.tile([128, 512], F32, tag="sacc", bufs=2)
                            nc.vector.tensor_copy(s_acc, pt[:, 0, :])
                            for kc in range(1, KC):
                                nc.vector.tensor_add(s_acc, s_acc, pt[:, kc, :])
                            ps_bc = psum.tile(
                                [128, 512], F32, name="ps_bc", tag="proj", bufs=2
                            )
                            nc.tensor.matmul(
                                ps_bc, lhsT=ones_sb, rhs=s_acc,
                                start=True, stop=True,
                            )
                        else:
                            ps_bc = ps_bcs[qg]
                        recip = tmp_pool.tile([128, 512], F32, tag="recip", bufs=2)
                        nc.vector.reciprocal_approx_fast(out=recip, in_=ps_bc)
                        ctxT_sb = out_pool.tile([128, 512], BF16, tag="ctxT")
                        nc.vector.tensor_mul(ctxT_sb, ps_cs[qg], recip)
                        t0 = tok0 + qg * 512
                        if n_slc >= 512:
                            dst = ctxT[
                                t0 // n_slc,
                                h * 128 : (h + 1) * 128,
                                t0 % n_slc : t0 % n_slc + 512,
                            ]
                            src = ctxT_sb[:, :]
                            nc.sync.dma_start(out=dst, in_=src)
                        else:
                            nj = 512 // n_slc
                            for jj in range(nj):
                                nc.sync.dma_start(
                                    out=ctxT[
                                        t0 // n_slc + jj,
                                        h * 128 : (h + 1) * 128,
                                        :,
                                    ],
                                    in_=ctxT_sb[
                                        :, jj * n_slc : (jj + 1) * n_slc
                                    ],
                                )

    nc.compile()
    nc.finalize()
    return nc


def build_outproj(n_cores=N_CORES, t_slc=T_SLC, lowering=True):
    """Kernel 2: out[t, o] = ctxT_all.T @ WoT for this core's token slice."""
    nc = bacc.Bacc(
        None,
        num_devices=n_cores,
        target_bir_lowering=lowering,
        enable_partition_id=False,
    )
    ctxa = nc.dram_tensor("ctxa", [H, t_slc], BF16, kind="ExternalInput")
    woT = nc.dram_tensor("woT", [H, H], BF16, kind="ExternalInput")
    out = nc.dram_tensor("out", [t_slc, H], F32, kind="ExternalOutput")

    TT = t_slc // 128  # 4
    OB = H // 512  # 4

    with tile.TileContext(nc) as tc, ExitStack() as ctx:
        wpool = ctx.enter_context(tc.tile_pool(name="wpool", bufs=2))
        cpool = ctx.enter_context(tc.tile_pool(name="cpool", bufs=1))
        tmp = ctx.enter_context(tc.tile_pool(name="tmp", bufs=4))
        psum = ctx.enter_context(tc.tile_pool(name="psum", bufs=2, space="PSUM"))

        ctxa_sb = cpool.tile([128, HC, t_slc], BF16)
        ctxa_v = ctxa.rearrange("(c p) t -> p c t", p=128)
        for fc in range(HC):
            nc.scalar.dma_start(out=ctxa_sb[:, fc, :], in_=ctxa_v[:, fc, :])
        woT_v = woT.rearrange("(c p) o -> p c o", p=128)

        # o-blocks outer so each WoT column block's DMA overlaps the previous
        # block's matmuls
        for ob in range(OB):
            osl = slice(ob * 512, (ob + 1) * 512)
            wo_sb = wpool.tile([128, HC, 512], BF16, name="wo_sb", tag="wo")
            nc.sync.dma_start(out=wo_sb, in_=woT_v[:, :, osl])
            for tt in range(TT):
                ps = psum.tile([128, 512], F32, tag="o", bufs=2)
                for fc in range(HC):
                    nc.tensor.matmul(
                        ps,
                        lhsT=ctxa_sb[:, fc, tt * 128 : (tt + 1) * 128],
                        rhs=wo_sb[:, fc, :],
                        start=(fc == 0),
                        stop=(fc == HC - 1),
                    )
                o_sb = tmp.tile([128, 512], F32, tag="osb")
                nc.vector.tensor_copy(o_sb, ps)
                nc.sync.dma_start(
                    out=out[tt * 128 : (tt + 1) * 128, osl],
                    in_=o_sb,
                )

    nc.compile()
    nc.finalize()
    return nc


def _prep_inputs(hidden_states, Wq, Wk, Wv, Wo, Aq, Bq, Av, Bv, position_ids):
    """Host-side layout prep: slice per core, transpose, cast, RoPE tables."""
    hs = np.ascontiguousarray(hidden_states.reshape(T, H).T).astype(np_bf16)
    woT = np.ascontiguousarray(Wo.T).astype(np_bf16)
    abT = np.ascontiguousarray(np.concatenate([Aq, Av], axis=0).T).astype(np_bf16)

    pos = np.asarray(position_ids).reshape(-1).astype(np.float64)  # [S]
    inv_freq = 1.0 / (
        ROPE_BASE ** (np.arange(0, HD, 2, dtype=np.float64) / HD)
    )  # [64]
    freqs = pos[:, None] * inv_freq[None, :]  # [S, 64]
    cos = np.cos(freqs).T.astype(np.float32)  # [64, S]
    sin = np.sin(freqs).T.astype(np.float32)
    cosT = np.concatenate([cos, cos], axis=0)  # [128, S]
    sinT = np.concatenate([-sin, sin], axis=0)  # sign-folded rotate_half

    per_core = []
    for c in range(N_CORES):
        fsl = slice(c * F_LOC, (c + 1) * F_LOC)
        per_core.append(
            dict(
                hsT=hs,
                wqT=np.ascontiguousarray(Wq[fsl, :].T).astype(np_bf16),
                wkT=np.ascontiguousarray(Wk[fsl, :].T).astype(np_bf16),
                wvT=np.ascontiguousarray(Wv[fsl, :].T).astype(np_bf16),
                abT=abT,
                bqT=np.ascontiguousarray(
                    (Bq[fsl, :] * LORA_SCALING).T
                ).astype(np_bf16),
                bvT=np.ascontiguousarray(
                    (Bv[fsl, :] * LORA_SCALING).T
                ).astype(np_bf16),
                cosT=cosT.astype(np_bf16),
                sinT=sinT.astype(np_bf16),
                woT=woT,
            )
        )
    return per_core


_CACHE = {}


def _get_compiled():
    if "fn" in _CACHE:
        return _CACHE["fn"]

    import jax
    from jax.sharding import Mesh, PartitionSpec as P
    from jax.experimental.shard_map import shard_map

    nc1 = build_attn()
    nc2 = build_outproj()

    attn_in = ["hsT", "wqT", "wkT", "wvT", "abT", "bqT", "bvT", "cosT", "sinT"]

    def f(hsT, wqT, wkT, wvT, abT, bqT, bvT, cosT, sinT, woT):
        (ctxT,) = bass2jax.bass_exec(
            (jax.core.ShapedArray((N_CORES, F_LOC, T // N_CORES), np_bf16),),
            tuple(attn_in),
            ("ctxT",),
            nc1,
            {},
            True,
            True,
            hsT,
            wqT,
            wkT,
            wvT,
            abT,
            bqT,
            bvT,
            cosT,
            sinT,
        )
        # exchange: already [8, 256, 512]; chunk j -> core j
        g = jax.lax.all_to_all(ctxT, "core", split_axis=0, concat_axis=0,
                               tiled=True)
        g = g.reshape(H, T_SLC)
        (out,) = bass2jax.bass_exec(
            (jax.core.ShapedArray((T_SLC, H), np.float32),),
            ("ctxa", "woT"),
            ("out",),
            nc2,
            {},
            True,
            True,
            g,
            woT,
        )
        return out

    import jax as _jax

    mesh = Mesh(np.asarray(_jax.devices()[:N_CORES]), ("core",))
    # hsT/abT/cosT/sinT/woT replicated; w*/b* weight shards per-core
    rep = {"hsT", "abT", "cosT", "sinT", "woT"}
    names = [
        "hsT", "wqT", "wkT", "wvT", "abT", "bqT", "bvT", "cosT", "sinT", "woT",
    ]
    specs_in = tuple(P() if n in rep else P("core") for n in names)
    fn = _jax.jit(
        shard_map(
            f, mesh=mesh, in_specs=specs_in, out_specs=P("core"), check_rep=False
        )
    )
    _CACHE["fn"] = fn
    _CACHE["names"] = names
    _CACHE["rep"] = rep
    return fn


def kernel(**inputs):
    fn = _get_compiled()
    per_core = _prep_inputs(**inputs)

    names, rep = _CACHE["names"], _CACHE["rep"]
    args = [
        per_core[0][n]
        if n in rep
        else np.concatenate([per_core[c][n] for c in range(N_CORES)], axis=0)
        for n in names
    ]
    out = fn(*args)
    res = np.asarray(out)  # [N_CORES * T_SLC, H] = [T, H]
    return res.reshape(B, S, H).astype(np.float32)


# revision 19
# speedup vs baseline: 44.9599x; 1.0007x over previous
"""Trainium2 Bass kernel for nn_LlamaAttention_17085379903943.

LlamaAttention with LoRA on q/v projections + RoPE, B=2, S=2048, H=2048,
nh=16, hd=128, LoRA rank 16.

Sharding: tensor-parallel over heads across 8 NeuronCores. Each core owns 2
heads (a 256-wide slice of the qkv projection output space) and computes
q/k/v projections (+LoRA), RoPE, and full-sequence softmax attention for its
heads over both batch elements. The per-core context output ctxT
[256, 4096] (features x tokens) is exchanged with a jax-level all_to_all so
that each core ends up with all 2048 context features for a 512-token slice,
then a second bass kernel applies the output projection.  Host code only
slices/transposes/casts inputs and concatenates the 8 output slices.

Compute layout notes (PE matmul computes out = lhsT.T @ rhs, contraction on
the partition dim):
 - hs is fed transposed (hsT [H, T]) so h sits on partitions for the
   projections.  q/k are produced transposed per head (qT/kT [hd, t]) which
   is exactly the layout attention needs; v is produced natural [t, d].
 - scoresT [k, q] = kT_chunk.T @ qT, exp via ScalarE (scale=1/sqrt(hd)
   folded in) with bf16 probsT output.
 - ctx [q, d] accumulates probsT_chunk.T @ [v | ones]: the appended ones
   column makes column 128 the softmax denominator for free; eviction
   multiplies by its reciprocal (per-partition scalar broadcast).
 - RoPE is applied on the [d, t] layout: the rotate-half partition shift is
   done with two small PSUM->SBUF DMAs, sign folded into the sin table.
"""

import math
from contextlib import ExitStack

import numpy as np
import ml_dtypes

import concourse.bass as bass
import concourse.mybir as mybir
import concourse.tile as tile
from concourse import bacc
from concourse import bass2jax
from concourse.masks import make_identity

N_CORES = 8
B = 2
S = 2048
H = 2048
NH = 16
HD = 128
R = 16
LORA_SCALING = 2.0
ROPE_BASE = 10000.0

H_LOC = NH // N_CORES  # heads per core = 2
F_LOC = H_LOC * HD  # feature slice per core = 256
HC = H // 128  # h chunks = 16
T = B * S  # tokens = 4096
T_SLC = T // N_CORES  # output token slice per core = 512

BF16 = mybir.dt.bfloat16
F32 = mybir.dt.float32
np_bf16 = ml_dtypes.bfloat16

AF = mybir.ActivationFunctionType

SUMS_MODE = "pe"  # "dve": DVE partial sums + one fp32 collapse MM; "pe": ones-matmul


def build_attn(s=S, n_cores=N_CORES, lowering=True):
    """Kernel 1: qkv projections + LoRA + RoPE + attention -> ctxT."""
    t_all = B * s
    TB = s // 512  # 512-wide t blocks per batch (proj moving blocks)
    TT = s // 128  # 128-wide t tiles per batch
    QG = s // 512  # 512-wide q groups
    KC = s // 128  # k chunks

    nc = bacc.Bacc(
        None,
        num_devices=n_cores,
        target_bir_lowering=lowering,
        enable_partition_id=False,
    )

    hsT = nc.dram_tensor("hsT", [H, t_all], BF16, kind="ExternalInput")
    wqT = nc.dram_tensor("wqT", [H, F_LOC], BF16, kind="ExternalInput")
    wkT = nc.dram_tensor("wkT", [H, F_LOC], BF16, kind="ExternalInput")
    wvT = nc.dram_tensor("wvT", [H, F_LOC], BF16, kind="ExternalInput")
    abT = nc.dram_tensor("abT", [H, 2 * R], BF16, kind="ExternalInput")
    bqT = nc.dram_tensor("bqT", [R, F_LOC], BF16, kind="ExternalInput")
    bvT = nc.dram_tensor("bvT", [R, F_LOC], BF16, kind="ExternalInput")
    cosT = nc.dram_tensor("cosT", [HD, s], BF16, kind="ExternalInput")
    sinT = nc.dram_tensor("sinT", [HD, s], BF16, kind="ExternalInput")
    n_slc = t_all // n_cores
    ctxT = nc.dram_tensor("ctxT", [n_cores, F_LOC, n_slc], BF16, kind="ExternalOutput")

    inv_sqrt_hd = 1.0 / math.sqrt(HD)

    with tile.TileContext(nc) as tc, ExitStack() as ctx:
        const = ctx.enter_context(tc.tile_pool(name="const", bufs=1))
        wpool = ctx.enter_context(tc.tile_pool(name="wpool", bufs=1))
        hs_pool = ctx.enter_context(tc.tile_pool(name="hs_pool", bufs=HC))
        qk_pool = ctx.enter_context(tc.tile_pool(name="qk_pool", bufs=1))
        v_pool = ctx.enter_context(tc.tile_pool(name="v_pool", bufs=1))
        lora_pool = ctx.enter_context(tc.tile_pool(name="lora_pool", bufs=1))
        probs_pool = ctx.enter_context(tc.tile_pool(name="probs_pool", bufs=2))
        tmp_pool = ctx.enter_context(tc.tile_pool(name="tmp_pool", bufs=3))
        out_pool = ctx.enter_context(tc.tile_pool(name="out_pool", bufs=4))
        psum = ctx.enter_context(tc.tile_pool(name="psum", bufs=1, space="PSUM"))

        # --- constants ---
        ident = const.tile([128, 128], BF16)
        make_identity(nc, ident)
        ones_sb = const.tile([128, 128], F32 if SUMS_MODE == "dve" else BF16)
        nc.vector.memset(ones_sb, 1.0)
        # --- weights (resident); tiny LoRA weights first so the very first
        # matmul accumulation (lora pass) can start as soon as hs chunk 0
        # lands; tables last (only needed at first RoPE eviction) ---
        ab_sb = wpool.tile([128, HC, 2 * R], BF16)
        nc.scalar.dma_start(out=ab_sb, in_=abT.rearrange("(c p) o -> p c o", p=128))
        bq_sb = wpool.tile([R, F_LOC], BF16)
        nc.scalar.dma_start(out=bq_sb, in_=bqT[:, :])
        bv_sb = wpool.tile([R, F_LOC], BF16)
        nc.scalar.dma_start(out=bv_sb, in_=bvT[:, :])
        wq_sb = wpool.tile([128, HC, F_LOC], BF16)
        nc.scalar.dma_start(out=wq_sb, in_=wqT.rearrange("(c p) o -> p c o", p=128))
        wk_sb = wpool.tile([128, HC, F_LOC], BF16)
        nc.scalar.dma_start(out=wk_sb, in_=wkT.rearrange("(c p) o -> p c o", p=128))
        wv_sb = wpool.tile([128, HC, F_LOC], BF16)
        nc.scalar.dma_start(out=wv_sb, in_=wvT.rearrange("(c p) o -> p c o", p=128))
        cos_sb = const.tile([HD, s], BF16)
        nc.scalar.dma_start(out=cos_sb, in_=cosT[:, :])
        sin_sb = const.tile([HD, s], BF16)
        nc.scalar.dma_start(out=sin_sb, in_=sinT[:, :])

        hsT_v = hsT.rearrange("(c p) t -> c p t", p=128)

        for b in range(B):
            tok0 = b * s

            # --- load hs chunks for this batch ---
            hs_tiles = []
            for c in range(HC):
                hst = hs_pool.tile([128, s], BF16, name=f"hst{c}", tag="hst")
                nc.sync.dma_start(out=hst, in_=hsT_v[c, :, tok0 : tok0 + s])
                hs_tiles.append(hst)

            # --- LoRA down-projection: [2R, s] = abT.T @ hsT ---
            lora_sb = lora_pool.tile([2 * R, s], BF16, tag="lora")
            lorav_sb = lora_pool.tile([R, s], BF16, tag="lorav")
            for tb in range(TB):
                ps_l = psum.tile([2 * R, 512], F32, tag="proj", bufs=2)
                for c in range(HC):
                    nc.tensor.matmul(
                        ps_l,
                        lhsT=ab_sb[:, c, :],
                        rhs=hs_tiles[c][:, tb * 512 : (tb + 1) * 512],
                        start=(c == 0),
                        stop=(c == HC - 1),
                    )
                nc.vector.tensor_copy(lora_sb[:, tb * 512 : (tb + 1) * 512], ps_l)
            nc.sync.dma_start(out=lorav_sb[:, :], in_=lora_sb[R : 2 * R, :])

            # --- q/k projections (+ q LoRA) with fused RoPE eviction ---
            qT_sb = qk_pool.tile([128, H_LOC, s], BF16, name="qT_sb", tag="qT")
            kT_sb = qk_pool.tile([128, H_LOC, s], BF16, name="kT_sb", tag="kT")
            for w_sb, b_lora_sb, dest in (
                (wq_sb, bq_sb, qT_sb),
                (wk_sb, None, kT_sb),
            ):
                for ot in range(H_LOC):
                    for tb in range(TB):
                        tsl = slice(tb * 512, (tb + 1) * 512)
                        ps = psum.tile([128, 512], F32, tag="proj", bufs=2)
                        for c in range(HC):
                            nc.tensor.matmul(
                                ps,
                                lhsT=w_sb[:, c, ot * 128 : (ot + 1) * 128],
                                rhs=hs_tiles[c][:, tsl],
                                start=(c == 0),
                                stop=(c == HC - 1 and b_lora_sb is None),
                            )
                        if b_lora_sb is not None:
                            nc.tensor.matmul(
                                ps,
                                lhsT=b_lora_sb[:, ot * 128 : (ot + 1) * 128],
                                rhs=lora_sb[0:R, tsl],
                                start=False,
                                stop=True,
                            )
                        # RoPE: dest = qf*cos + shift(qf)*sin'  (sign in table)
                        qf = tmp_pool.tile([128, 512], F32, tag="qf", bufs=2)
                        nc.vector.tensor_copy(qf, ps)
                        shift = tmp_pool.tile([128, 512], F32, tag="shift", bufs=2)
                        nc.sync.dma_start(out=shift[0:64, :], in_=qf[64:128, :])
                        nc.sync.dma_start(out=shift[64:128, :], in_=qf[0:64, :])
                        t1 = tmp_pool.tile([128, 512], F32, tag="t1", bufs=2)
                        nc.vector.tensor_mul(t1, shift, sin_sb[:, tsl])
                        t2 = tmp_pool.tile([128, 512], F32, tag="t2", bufs=2)
                        nc.vector.tensor_mul(t2, qf, cos_sb[:, tsl])
                        nc.vector.tensor_add(dest[:, ot, tsl], t1, t2)

            # --- v projection, transposed orientation [o, t] then PE
            #     transpose to v_sb [t, d] (ctx stationary layout) ---
            vT_sb = qk_pool.tile([128, H_LOC, s], BF16, name="vT_sb", tag="vT")
            for ot in range(H_LOC):
                for tb in range(TB):
                    tsl = slice(tb * 512, (tb + 1) * 512)
                    ps = psum.tile([128, 512], F32, tag="proj", bufs=2)
                    for c in range(HC):
                        nc.tensor.matmul(
                            ps,
                            lhsT=wv_sb[:, c, ot * 128 : (ot + 1) * 128],
                            rhs=hs_tiles[c][:, tsl],
                            start=(c == 0),
                            stop=False,
                        )
                    nc.tensor.matmul(
                        ps,
                        lhsT=bv_sb[:, ot * 128 : (ot + 1) * 128],
                        rhs=lorav_sb[0:R, tsl],
                        start=False,
                        stop=True,
                    )
                    nc.vector.tensor_copy(vT_sb[:, ot, tsl], ps)
            v_sb = v_pool.tile([128, TT, H_LOC, HD], BF16, name="v_sb", tag="v")
            for h in range(H_LOC):
                for tt in range(TT):
                    ps_t = psum.tile([128, 128], BF16, tag="small", bufs=2)
                    nc.tensor.transpose(
                        ps_t, vT_sb[:, h, tt * 128 : (tt + 1) * 128], ident
                    )
                    nc.vector.tensor_copy(v_sb[:, tt, h, :], ps_t)

            # --- attention per head: scores/probs in [k, q], ctx in [d, q] ---
            for h in range(H_LOC):
                for pair_qgs in [list(range(p, min(p + 2, QG)))
                                 for p in range(0, QG, 2)]:
                    pts = []
                    for qg in pair_qgs:
                        qsl = slice(qg * 512, (qg + 1) * 512)
                        pt = probs_pool.tile(
                            [128, KC, 512], BF16, name=f"pt{qg % 2}", tag="probs"
                        )
                        pts.append((qg, qsl, pt))
                        for kc in range(KC):
                            ps_s = psum.tile([128, 512], F32, tag="small", bufs=2)
                            nc.tensor.matmul(
                                ps_s,
                                lhsT=kT_sb[:, h, kc * 128 : (kc + 1) * 128],
                                rhs=qT_sb[:, h, qsl],
                                start=True,
                                stop=True,
                            )
                            nc.scalar.activation(
                                pt[:, kc, :], ps_s, AF.Exp, scale=inv_sqrt_hd
                            )
                    # ctx accumulation, v chunk stationary, probsT moving
                    ps_cs = {}
                    ps_bcs = {}
                    for qg, _, _ in pts:
                        ps_cs[qg] = psum.tile(
                            [128, 512], F32, name=f"ps_c{qg % 2}", tag="ctx", bufs=2
                        )
                        if SUMS_MODE == "pe":
                            ps_bcs[qg] = psum.tile(
                                [128, 512], F32, name=f"ps_b{qg % 2}",
                                tag="proj", bufs=2,
                            )
                    for kc in range(KC):
                        for qg, _, pt in pts:
                            nc.tensor.matmul(
                                ps_cs[qg],
                                lhsT=v_sb[:, kc, h, :],
                                rhs=pt[:, kc, :],
                                start=(kc == 0),
                                stop=(kc == KC - 1),
                            )
                        if SUMS_MODE == "pe":
                            for qg, _, pt in pts:
                                nc.tensor.matmul(
                                    ps_bcs[qg],
                                    lhsT=ones_sb,
                                    rhs=pt[:, kc, :],
                                    start=(kc == 0),
                                    stop=(kc == KC - 1),
                                )
                    for qg, qsl, pt in pts:
                        if SUMS_MODE == "dve":
                            # f32 partial sums over the 16 k chunks, then a
                            # single all-ones fp32 matmul collapses partitions
                            # AND broadcasts the total to every row.
                            s_acc = tmp_pool.tile([128, 512], F32, tag="sacc", bufs=2)
                            nc.vector.tensor_copy(s_acc, pt[:, 0, :])
                            for kc in range(1, KC):
                                nc.vector.tensor_add(s_acc, s_acc, pt[:, kc, :])
                            ps_bc = psum.tile(
                                [128, 512], F32, name="ps_bc", tag="proj", bufs=2
                            )
                            nc.tensor.matmul(
                                ps_bc, lhsT=ones_sb, rhs=s_acc,
                                start=True, stop=True,
                            )
                        else:
                            ps_bc = ps_bcs[qg]
                        recip = tmp_pool.tile([128, 512], F32, tag="recip", bufs=2)
                        nc.vector.reciprocal_approx_fast(out=recip, in_=ps_bc)
                        ctxT_sb = out_pool.tile([128, 512], BF16, tag="ctxT")
                        nc.vector.tensor_mul(ctxT_sb, ps_cs[qg], recip)
                        t0 = tok0 + qg * 512
                        if n_slc >= 512:
                            dst = ctxT[
                                t0 // n_slc,
                                h * 128 : (h + 1) * 128,
                                t0 % n_slc : t0 % n_slc + 512,
                            ]
                            src = ctxT_sb[:, :]
                            nc.sync.dma_start(out=dst, in_=src)
                        else:
                            nj = 512 // n_slc
                            for jj in range(nj):
                                nc.sync.dma_start(
                                    out=ctxT[
                                        t0 // n_slc + jj,
                                        h * 128 : (h + 1) * 128,
                                        :,
                                    ],
                                    in_=ctxT_sb[
                                        :, jj * n_slc : (jj + 1) * n_slc
                                    ],
                                )

    nc.compile()
    nc.finalize()
    return nc


def build_outproj(n_cores=N_CORES, t_slc=T_SLC, lowering=True):
    """Kernel 2: out[t, o] = ctxT_all.T @ WoT for this core's token slice."""
    nc = bacc.Bacc(
        None,
        num_devices=n_cores,
        target_bir_lowering=lowering,
        enable_partition_id=False,
    )
    ctxa = nc.dram_tensor("ctxa", [H, t_slc], BF16, kind="ExternalInput")
    woT = nc.dram_tensor("woT", [H, H], BF16, kind="ExternalInput")
    out = nc.dram_tensor("out", [t_slc, H], F32, kind="ExternalOutput")

    TT = t_slc // 128  # 4
    OB = H // 512  # 4

    with tile.TileContext(nc) as tc, ExitStack() as ctx:
        wpool = ctx.enter_context(tc.tile_pool(name="wpool", bufs=2))
        cpool = ctx.enter_context(tc.tile_pool(name="cpool", bufs=1))
        tmp = ctx.enter_context(tc.tile_pool(name="tmp", bufs=4))
        psum = ctx.enter_context(tc.tile_pool(name="psum", bufs=2, space="PSUM"))

        ctxa_sb = cpool.tile([128, HC, t_slc], BF16)
        ctxa_v = ctxa.rearrange("(c p) t -> p c t", p=128)
        for fc in range(HC):
            nc.scalar.dma_start(out=ctxa_sb[:, fc, :], in_=ctxa_v[:, fc, :])
        woT_v = woT.rearrange("(c p) o -> p c o", p=128)

        # o-blocks outer so each WoT column block's DMA overlaps the previous
        # block's matmuls
        for ob in range(OB):
            osl = slice(ob * 512, (ob + 1) * 512)
            wo_sb = wpool.tile([128, HC, 512], BF16, name="wo_sb", tag="wo")
            nc.sync.dma_start(out=wo_sb, in_=woT_v[:, :, osl])
            for tt in range(TT):
                ps = psum.tile([128, 512], F32, tag="o", bufs=2)
                for fc in range(HC):
                    nc.tensor.matmul(
                        ps,
                        lhsT=ctxa_sb[:, fc, tt * 128 : (tt + 1) * 128],
                        rhs=wo_sb[:, fc, :],
                        start=(fc == 0),
                        stop=(fc == HC - 1),
                    )
                o_sb = tmp.tile([128, 512], F32, tag="osb")
                nc.vector.tensor_copy(o_sb, ps)
                nc.sync.dma_start(
                    out=out[tt * 128 : (tt + 1) * 128, osl],
                    in_=o_sb,
                )

    nc.compile()
    nc.finalize()
    return nc


def _prep_inputs(hidden_states, Wq, Wk, Wv, Wo, Aq, Bq, Av, Bv, position_ids):
    """Host-side layout prep: slice per core, transpose, cast, RoPE tables."""
    hs = np.ascontiguousarray(hidden_states.reshape(T, H).T).astype(np_bf16)
    woT = np.ascontiguousarray(Wo.T).astype(np_bf16)
    abT = np.ascontiguousarray(np.concatenate([Aq, Av], axis=0).T).astype(np_bf16)

    pos = np.asarray(position_ids).reshape(-1).astype(np.float64)  # [S]
    inv_freq = 1.0 / (
        ROPE_BASE ** (np.arange(0, HD, 2, dtype=np.float64) / HD)
    )  # [64]
    freqs = pos[:, None] * inv_freq[None, :]  # [S, 64]
    cos = np.cos(freqs).T.astype(np.float32)  # [64, S]
    sin = np.sin(freqs).T.astype(np.float32)
    cosT = np.concatenate([cos, cos], axis=0)  # [128, S]
    sinT = np.concatenate([-sin, sin], axis=0)  # sign-folded rotate_half

    per_core = []
    for c in range(N_CORES):
        fsl = slice(c * F_LOC, (c + 1) * F_LOC)
        per_core.append(
            dict(
                hsT=hs,
                wqT=np.ascontiguousarray(Wq[fsl, :].T).astype(np_bf16),
                wkT=np.ascontiguousarray(Wk[fsl, :].T).astype(np_bf16),
                wvT=np.ascontiguousarray(Wv[fsl, :].T).astype(np_bf16),
                abT=abT,
                bqT=np.ascontiguousarray(
                    (Bq[fsl, :] * LORA_SCALING).T
                ).astype(np_bf16),
                bvT=np.ascontiguousarray(
                    (Bv[fsl, :] * LORA_SCALING).T
                ).astype(np_bf16),
                cosT=cosT.astype(np_bf16),
                sinT=sinT.astype(np_bf16),
                woT=woT,
            )
        )
    return per_core


_CACHE = {}


def _get_compiled():
    if "fn" in _CACHE:
        return _CACHE["fn"]

    import jax
    from jax.sharding import Mesh, PartitionSpec as P
    from jax.experimental.shard_map import shard_map

    nc1 = build_attn()
    nc2 = build_outproj()

    attn_in = ["hsT", "wqT", "wkT", "wvT", "abT", "bqT", "bvT", "cosT", "sinT"]

    def f(hsT, wqT, wkT, wvT, abT, bqT, bvT, cosT, sinT, woT):
        (ctxT,) = bass2jax.bass_exec(
            (jax.core.ShapedArray((N_CORES, F_LOC, T // N_CORES), np_bf16),),
            tuple(attn_in),
            ("ctxT",),
            nc1,
            {},
            True,
            True,
            hsT,
            wqT,
            wkT,
            wvT,
            abT,
            bqT,
            bvT,
            cosT,
            sinT,
        )
        # exchange: already [8, 256, 512]; chunk j -> core j
        g = jax.lax.all_to_all(ctxT, "core", split_axis=0, concat_axis=0,
                               tiled=True)
        g = g.reshape(H, T_SLC)
        (out,) = bass2jax.bass_exec(
            (jax.core.ShapedArray((T_SLC, H), np.float32),),
            ("ctxa", "woT"),
            ("out",),
            nc2,
            {},
            True,
            True,
            g,
            woT,
        )
        return out

    import jax as _jax

    mesh = Mesh(np.asarray(_jax.devices()[:N_CORES]), ("core",))
    # hsT/abT/cosT/sinT/woT replicated; w*/b* weight shards per-core
    rep = {"hsT", "abT", "cosT", "sinT", "woT"}
    names = [
        "hsT", "wqT", "wkT", "wvT", "abT", "bqT", "bvT", "cosT", "sinT", "woT",
    ]
    specs_in = tuple(P() if n in rep else P("core") for n in names)
    fn = _jax.jit(
        shard_map(
            f, mesh=mesh, in_specs=specs_in, out_specs=P("core"), check_rep=False
        )
    )
    _CACHE["fn"] = fn
    _CACHE["names"] = names
    _CACHE["rep"] = rep
    return fn


def kernel(**inputs):
    fn = _get_compiled()
    per_core = _prep_inputs(**inputs)

    names, rep = _CACHE["names"], _CACHE["rep"]
    args = [
        per_core[0][n]
        if n in rep
        else np.concatenate([per_core[c][n] for c in range(N_CORES)], axis=0)
        for n in names
    ]
    out = fn(*args)
    res = np.asarray(out)  # [N_CORES * T_SLC, H] = [T, H]
    return res.reshape(B, S, H).astype(np.float32)
